# revision 8
# baseline (speedup 1.0000x reference)
"""GCN encoder (2x GCNConv + BatchNorm + ReLU) on 8 Trainium2 NeuronCores.

Strategy (graph/data parallel, per sharding hint):
- Nodes are sharded across the 8 cores; each core owns 49 "windows" of 128
  destination nodes.  Source nodes are split into a "lo" half (owned by cores
  0-3) and "hi" half (cores 4-7); the half assignment is optimized on the host
  (discrepancy balancing) so each destination's in-edges split ~evenly, which
  makes the per-window gather-chunk maxima tight (low padding waste).
- norm factorizes: norm(s,d) = dis[s]*dis[d].  Source scaling dis[s] is folded
  into the feature tables (h~ = dis * h); destination scaling dis[d] is applied
  on eviction.  Messages aggregate with a constant identity matmul into PSUM.
- Gathers use the int16 dma_gather embedding path; each window has an "A"
  segment (sources in the half this core's HBM-pair parity built) and a "B"
  segment (other half); pad slots point at an all-zero row.
- Phase 1 (h1 = dis*(x @ W1)) is split across HBM-pair cores: the pair shares
  one h1 table (addr_space="Shared"); the even core computes/writes the lo
  half, the odd core the hi half (rank-dependent write offsets via
  partition_id + DynSlice).  A 2-core AllReduce barrier syncs the pair; it is
  hidden behind conv1's "A" pass, which only reads the self-built half.
- h2 = relu(bn1(conv1)) @ W2 is computed per-shard; an AllGather replicates
  the h2 table for conv2.  BatchNorm stats use E[x^2]-mean^2 via ones-vector
  matmuls accumulated in PSUM, then AllReduce.  b1/b2 cancel in BN.
"""

import sys

sys.path.insert(0, "/opt/trn_rl_repo")

import numpy as np

N_CORES = 8
P = 128
EPS = 1e-5

_FULL_CFG = dict(N=50000, IN=512, D1=256, D2=128)


# ---------------------------------------------------------------- host preprocessing

def _balance_halves(S, D, deg, N, seed=0):
    """Assign each node to the lo (+1) or hi (-1) half so that every dst's
    in-edges split ~evenly between halves.  Greedy discrepancy minimization."""
    order_s = np.argsort(S, kind="stable")
    Ds = D[order_s]
    starts = np.searchsorted(S[order_s], np.arange(N + 1))
    outdeg = np.diff(starts)
    rng = np.random.default_rng(seed)
    h = np.where(rng.random(N) < 0.5, 1, -1)

    for r in range(120):
        cur = np.bincount(D, weights=h[S].astype(np.float64), minlength=N)
        s_cursum = np.add.reduceat(cur[Ds], starts[:-1]) if len(Ds) else np.zeros(N)
        s_cursum[outdeg == 0] = 0
        gain = h * s_cursum - outdeg
        batch = 3000 if r < 10 else (800 if r < 40 else 250)
        lo_c = np.flatnonzero((gain > 0) & (h == 1))
        hi_c = np.flatnonzero((gain > 0) & (h == -1))
        nlo = int((h == 1).sum())
        k_lo = min(len(lo_c), batch + max(0, nlo - N // 2))
        k_hi = min(len(hi_c), batch + max(0, N // 2 - nlo))
        if k_lo + k_hi == 0:
            break
        h[lo_c[np.argsort(-gain[lo_c])[:k_lo]]] = -1
        h[hi_c[np.argsort(-gain[hi_c])[:k_hi]]] = 1

    cur = np.bincount(D, weights=h[S].astype(np.float64), minlength=N).astype(np.int64)
    in_order = np.argsort(D, kind="stable")
    Sin = S[in_order]
    in_starts = np.searchsorted(D[in_order], np.arange(N + 1))
    for sweep in range(6):
        bad = np.flatnonzero(np.abs(cur) >= 3)
        bad = bad[np.argsort(-np.abs(cur[bad]))]
        if len(bad) == 0:
            break
        for d in bad:
            cd = cur[d]
            if abs(cd) < 3:
                continue
            sign = 1 if cd > 0 else -1
            nbrs = Sin[in_starts[d]:in_starts[d + 1]]
            cands = nbrs[h[nbrs] == sign]
            if len(cands) == 0:
                continue
            best, bestg = -1, -(10 ** 9)
            for s in cands[:12]:
                od = Ds[starts[s]:starts[s + 1]]
                g = h[s] * cur[od].sum() - len(od)
                if g > bestg:
                    best, bestg = s, g
            od = Ds[starts[best]:starts[best + 1]]
            cur[od] -= 2 * h[best]
            h[best] = -h[best]
    # exact 50/50 split
    nlo = int((h == 1).sum())
    if nlo != N // 2:
        d = 1 if nlo > N // 2 else -1
        side = np.flatnonzero(h == d)
        s_cursum = np.add.reduceat(cur[Ds], starts[:-1]) if len(Ds) else np.zeros(N)
        s_cursum[outdeg == 0] = 0
        gain = h * s_cursum - outdeg
        take = side[np.argsort(-gain[side])[: abs(nlo - N // 2)]]
        for s in take:
            od = Ds[starts[s]:starts[s + 1]]
            cur[od] -= 2 * h[s]
            h[s] = -h[s]
    return h


def _preprocess(edge_index, N):
    """Graph preprocessing: half balancing, node permutation, A/B segment
    chunk assignment, gather indices.  Pure integer work on the host."""
    src = np.asarray(edge_index[0], dtype=np.int64)
    dst = np.asarray(edge_index[1], dtype=np.int64)
    loop = np.arange(N, dtype=np.int64)
    S = np.concatenate([src, loop])
    D = np.concatenate([dst, loop])

    deg = np.bincount(D, minlength=N)  # >= 1 (self loop)
    dis = (1.0 / np.sqrt(deg.astype(np.float64))).astype(np.float32)

    real_pc = N // N_CORES
    WPC = (real_pc + P - 1) // P          # windows per core
    SLOTS = WPC * P                        # slot positions per core
    BLK = SLOTS + 1                        # +1 trailing zero row per core block

    # half assignment (lo = cores 0-3), then deal each half by degree
    hsplit = _balance_halves(S, D, deg, N)
    core_of = np.empty(N, dtype=np.int64)
    for half, hv in ((0, 1), (1, -1)):
        nodes = np.flatnonzero(hsplit == hv)
        o = nodes[np.argsort(-deg[nodes], kind="stable")]
        core_of[o] = half * 4 + np.arange(len(o)) % 4

    half_node = core_of >= (N_CORES // 2)  # True = hi half
    halfE = half_node[S]
    deg_lo = np.bincount(D[~halfE], minlength=N)
    deg_hi = deg - deg_lo

    # position within core: sort by max(lo,hi) desc (then total) — with the
    # balanced halves lo≈hi, this keeps BOTH per-window maxima tight
    pos = np.empty(N, dtype=np.int64)
    node_by_cp = np.full((N_CORES, SLOTS), -1, dtype=np.int64)
    for c in range(N_CORES):
        nodes_c = np.flatnonzero(core_of == c)
        key = np.maximum(deg_lo[nodes_c], deg_hi[nodes_c])
        o = np.lexsort((-(deg_lo[nodes_c] + deg_hi[nodes_c]), -key))
        snodes = nodes_c[o]
        pos[snodes] = np.arange(len(snodes))
        node_by_cp[c, : len(snodes)] = snodes

    # per-core per-window maxima of lo/hi degree
    dlo_cp = np.zeros((N_CORES, SLOTS), dtype=np.int64)
    dhi_cp = np.zeros((N_CORES, SLOTS), dtype=np.int64)
    m = node_by_cp >= 0
    dlo_cp[m] = deg_lo[node_by_cp[m]]
    dhi_cp[m] = deg_hi[node_by_cp[m]]
    WL = dlo_cp.reshape(N_CORES, WPC, P).max(axis=2)  # [core, w]
    WH = dhi_cp.reshape(N_CORES, WPC, P).max(axis=2)
    ev = np.arange(N_CORES) % 2 == 0
    # segment A = the half this core's pair-parity built (even: lo, odd: hi)
    NA = np.maximum(WL[ev].max(axis=0), WH[~ev].max(axis=0))
    NB = np.maximum(WH[ev].max(axis=0), WL[~ev].max(axis=0))

    # idx segment offsets: per window [A seg][B seg], chunk-major inside
    seg = (NA + NB) * P
    base = np.concatenate([[0], np.cumsum(seg)])
    offA = base[:-1]
    offB = base[:-1] + NA * P
    TOT = int(base[-1])

    # edge -> segment: A iff src half == dst-core's parity half
    cD = core_of[D]
    inA = halfE == (cD % 2 == 1)
    key = D * 2 + (~inA).astype(np.int64)
    ksort = np.argsort(key, kind="stable")
    skey = key[ksort]
    starts = np.concatenate([[0], np.flatnonzero(np.diff(skey)) + 1])
    group_len = np.diff(np.concatenate([starts, [len(skey)]]))
    chunk_sorted = np.arange(len(skey)) - np.repeat(starts, group_len)
    chunk = np.empty(len(S), dtype=np.int64)
    chunk[ksort] = chunk_sorted

    wD = pos[D] // P
    slotD = pos[D] % P
    rel = (core_of[S] % (N_CORES // 2)) * BLK + pos[S]  # within-half row
    assert rel.max() < 32768
    epos = np.where(inA, offA[wD], offB[wD]) + chunk * P + slotD

    PADIDX = SLOTS  # block 0's trailing zero row (within-half view)
    flat = np.full(N_CORES * TOT, PADIDX, dtype=np.int16)
    flat[cD * TOT + epos] = rel.astype(np.int16)
    flat = flat.reshape(N_CORES, TOT)
    # wrap: idx i -> [i%16, i//16], replicated across the 8 groups of 16 rows
    wrapped16 = flat.reshape(N_CORES, TOT // 16, 16).transpose(0, 2, 1)
    idx_wrapped = np.tile(wrapped16, (1, P // 16, 1))  # [cores, 128, TOT/16]

    # per-core dis (by slot), 1.0 for dummies
    dis_cp = np.ones((N_CORES, SLOTS), dtype=np.float32)
    dis_cp[m] = dis[node_by_cp[m]]
    dismy = dis_cp.reshape(N_CORES, WPC, P).transpose(0, 2, 1)  # [c, 128, WPC]

    NTILES = N_CORES * WPC

    # stats mask: last window has (SLOTS - real_pc) dummy rows at the end
    n_dummy = SLOTS - real_pc
    statmask = np.ones((P, 2), dtype=np.float32)
    if n_dummy:
        statmask[P - n_dummy:, 1] = 0.0

    waste = float(seg.sum()) / max(1, len(S) / N_CORES) - 1.0
    return dict(
        WPC=WPC, SLOTS=SLOTS, BLK=BLK, NTILES=NTILES,
        NA=NA.astype(int), NB=NB.astype(int), TOT=TOT,
        offA=offA, offB=offB,
        idx_wrapped=idx_wrapped, dismy=dismy,
        statmask=statmask, node_by_cp=node_by_cp, pos=pos, core_of=core_of,
        dis=dis, real_pc=real_pc, waste=waste,
    )


def _pack_inputs(x, W1, W2, pp, cfg):
    """Build the device input arrays.  xb is the full permuted/prescaled x in
    table order; each core receives only the half it builds in phase 1."""
    import ml_dtypes

    bf16 = ml_dtypes.bfloat16
    N, IN, D1, D2 = cfg["N"], cfg["IN"], cfg["D1"], cfg["D2"]
    SLOTS, NTILES = pp["SLOTS"], pp["NTILES"]
    KC = IN // P

    xperm = np.zeros((N_CORES * SLOTS, IN), dtype=np.float32)
    m = pp["node_by_cp"] >= 0
    xperm[np.flatnonzero(m.reshape(-1))] = (
        x[pp["node_by_cp"][m]] * pp["dis"][pp["node_by_cp"][m]][:, None]
    )
    xb = (
        xperm.reshape(NTILES, P, KC, P)   # [b, j, kc, p]
        .transpose(0, 3, 2, 1)            # [b, p, kc, j]
        .astype(bf16)
    )
    w1b = W1.reshape(KC, P, D1).transpose(1, 0, 2).astype(bf16)   # [p, kc, D1]
    w2b = W2.reshape(D1 // P, P, D2).transpose(1, 0, 2).astype(bf16)  # [p, kc, D2]
    return xb, w1b, w2b


# ---------------------------------------------------------------- device kernel

def _build_kernel(cfg, pp, phases=5):
    import concourse.bacc as bacc
    import concourse.mybir as mybir
    import concourse.tile as tile
    from concourse.masks import make_identity
    from concourse.bass import ds
    from contextlib import ExitStack

    N, IN, D1, D2 = cfg["N"], cfg["IN"], cfg["D1"], cfg["D2"]
    WPC, SLOTS, BLK, NTILES = pp["WPC"], pp["SLOTS"], pp["BLK"], pp["NTILES"]
    NA, NB, TOT = pp["NA"], pp["NB"], pp["TOT"]
    offA, offB = pp["offA"], pp["offB"]
    KC = IN // P
    KC2 = D1 // P
    HB = (N_CORES // 2) * BLK        # rows per half
    NROWS = N_CORES * BLK            # table rows
    NBLK = N_CORES // 2              # blocks built per core (its parity half)
    NTILES2 = NBLK * WPC             # xb tiles per core
    NAmax = int(NA.max())
    NBmax = int(NB.max())
    RG = [list(range(N_CORES))]
    PAIR_RG = [[2 * k, 2 * k + 1] for k in range(N_CORES // 2)]
    f32, bf16, i16 = mybir.dt.float32, mybir.dt.bfloat16, mybir.dt.int16
    AF = mybir.ActivationFunctionType
    ALU = mybir.AluOpType

    nc = bacc.Bacc(num_devices=N_CORES)

    # ---- I/O
    xb_d = nc.dram_tensor("xb", [NTILES2, P, KC, P], bf16, kind="ExternalInput")
    w1_d = nc.dram_tensor("w1b", [P, KC, D1], bf16, kind="ExternalInput")
    w2_d = nc.dram_tensor("w2b", [P, KC2, D2], bf16, kind="ExternalInput")
    idx_d = nc.dram_tensor("idx", [P, TOT // 16], i16, kind="ExternalInput")
    dismy_d = nc.dram_tensor("dismy", [P, WPC], f32, kind="ExternalInput")
    mask_d = nc.dram_tensor("statmask", [P, 2], f32, kind="ExternalInput")
    g1_d = nc.dram_tensor("gamma1", [1, D1], f32, kind="ExternalInput")
    b1_d = nc.dram_tensor("beta1", [1, D1], f32, kind="ExternalInput")
    g2_d = nc.dram_tensor("gamma2", [1, D2], f32, kind="ExternalInput")
    b2_d = nc.dram_tensor("beta2", [1, D2], f32, kind="ExternalInput")
    out_d = nc.dram_tensor("out", [SLOTS, D2], f32, kind="ExternalOutput")

    # ---- internal DRAM
    # h1tab is shared within an HBM core pair: the even core writes the lo
    # half, the odd core the hi half.
    h1tab = nc.dram_tensor("h1tab", [NROWS, D1], bf16, kind="Internal",
                           addr_space="Shared")
    h2shard = nc.dram_tensor("h2shard", [BLK, D2], bf16, kind="Internal")
    h2tab = nc.dram_tensor("h2tab", [NROWS, D2], bf16, kind="Internal", addr_space="Shared")
    bar_in = nc.dram_tensor("bar_in", [1, 4], bf16, kind="Internal")
    bar_out = nc.dram_tensor("bar_out", [1, 4], bf16, kind="Internal")
    ar1_in = nc.dram_tensor("ar1_in", [1, 2 * D1], f32, kind="Internal")
    ar1_out = nc.dram_tensor("ar1_out", [1, 2 * D1], f32, kind="Internal", addr_space="Shared")
    ar2_in = nc.dram_tensor("ar2_in", [1, 2 * D2], f32, kind="Internal")
    ar2_out = nc.dram_tensor("ar2_out", [1, 2 * D2], f32, kind="Internal", addr_space="Shared")

    import concourse.bass as bass

    def pad_rows_ap(tensor, Dd):
        # rows {c*BLK + SLOTS : c in 0..7} of a [NROWS, Dd] table
        return bass.AP(tensor, SLOTS * Dd, [[BLK * Dd, N_CORES], [1, Dd]])

    with tile.TileContext(nc) as tc:
        es = ExitStack()
        with es:
            parity = nc.gpsimd.partition_id() % 2

            cpool = es.enter_context(tc.tile_pool(name="const", bufs=1))
            ident_b = cpool.tile([P, P], bf16)
            make_identity(nc, ident_b[:])
            ident_f = cpool.tile([P, P], f32)
            make_identity(nc, ident_f[:])
            w1_s = cpool.tile([P, KC, D1], bf16)
            nc.sync.dma_start(out=w1_s[:], in_=w1_d[:, :, :])
            w2_s = cpool.tile([P, KC2, D2], bf16)
            nc.sync.dma_start(out=w2_s[:], in_=w2_d[:, :, :])
            dismy_s = cpool.tile([P, WPC], f32)
            nc.sync.dma_start(out=dismy_s[:], in_=dismy_d[:, :])
            mask_s = cpool.tile([P, 2], f32)
            nc.sync.dma_start(out=mask_s[:], in_=mask_d[:, :])
            mask_b = cpool.tile([P, 2], bf16)
            nc.vector.tensor_copy(out=mask_b[:], in_=mask_s[:])
            idx_s = cpool.tile([P, TOT // 16], i16)
            nc.sync.dma_start(out=idx_s[:], in_=idx_d[:, :])
            gb_s = cpool.tile([1, 2 * D1 + 2 * D2], f32)  # gamma1|beta1|gamma2|beta2
            nc.sync.dma_start(out=gb_s[:, 0:D1], in_=g1_d[:, :])
            nc.sync.dma_start(out=gb_s[:, D1: 2 * D1], in_=b1_d[:, :])
            nc.sync.dma_start(out=gb_s[:, 2 * D1: 2 * D1 + D2], in_=g2_d[:, :])
            nc.sync.dma_start(out=gb_s[:, 2 * D1 + D2:], in_=b2_d[:, :])

            # zero ALL pad rows (both pair cores write identical zeros: benign)
            zrow = cpool.tile([N_CORES, D1], bf16)
            nc.vector.memset(zrow[:], 0)
            nc.gpsimd.dma_start(out=pad_rows_ap(h1tab, D1), in_=zrow[:])

            # ---------------- phase 1: my parity half of h1tab ----------------
            with (
                tc.tile_pool(name="p1x", bufs=6) as xpool,
                tc.tile_pool(name="p1s", bufs=2) as spool,
                tc.tile_pool(name="p1p", bufs=4, space="PSUM") as ppool1,
            ):
                XB = 7 if WPC % 7 == 0 else 1   # x tiles per DMA
                for blk in range(NBLK):
                    stage = spool.tile([P, WPC, D1], bf16, tag="stage")
                    for tb in range(WPC // XB):
                        b0 = blk * WPC + tb * XB
                        xt = xpool.tile([P, XB, KC, P], bf16, tag="xt")
                        nc.sync.dma_start(
                            out=xt[:], in_=xb_d[b0: b0 + XB].rearrange("b p k j -> p b k j")
                        )
                        for t2 in range(XB):
                            t = tb * XB + t2
                            ps = ppool1.tile([P, D1], f32, tag="ps1")
                            for kc in range(KC):
                                nc.tensor.matmul(
                                    out=ps[:], lhsT=xt[:, t2, kc, :], rhs=w1_s[:, kc, :],
                                    start=(kc == 0), stop=(kc == KC - 1),
                                )
                            nc.scalar.activation(
                                stage[:, t, :], ps[:], AF.Copy
                            )
                    nc.gpsimd.dma_start(
                        out=h1tab[ds(parity * HB + blk * BLK, SLOTS), :].rearrange(
                            "(t p) d -> p t d", p=P
                        ),
                        in_=stage[:],
                    )

            if phases < 2:
                outst = cpool.tile([P, WPC, D2], f32)
                nc.vector.memset(outst[:], 0)
                nc.gpsimd.dma_start(
                    out=out_d[0:SLOTS, :].rearrange("(t p) d -> p t d", p=P),
                    in_=outst[:],
                )
                return nc

            # ---------------- conv1 A pass: my-half gathers (no barrier) -----
            mid_es = ExitStack()
            o1_pool = mid_es.enter_context(tc.tile_pool(name="o1", bufs=1))
            o1A = o1_pool.tile([P, WPC, D1], bf16)     # A partials
            o1_all = o1_pool.tile([P, WPC, D1], bf16)  # combined conv1 out
            viewA1 = h1tab[ds(parity * HB, HB), :]
            viewB1 = h1tab[ds(HB - parity * HB, HB), :]
            with (
                tc.tile_pool(name="gA", bufs=3) as gpoolA,
                tc.tile_pool(name="cAp", bufs=3, space="PSUM") as wpoolA,
            ):
                for w in range(WPC):
                    na = int(NA[w])
                    gb = gpoolA.tile([P, NAmax, D1], bf16, tag="gA")
                    nc.gpsimd.dma_gather(
                        gb[:, 0:na, :], viewA1,
                        idx_s[:, offA[w] // 16: offA[w] // 16 + na * 8],
                        na * P, na * P, D1, elem_step=D1, single_packet=False,
                    )
                    ps = wpoolA.tile([P, D1], f32, tag="winA")
                    for j in range(na):
                        nc.tensor.matmul(
                            out=ps[:], lhsT=ident_b[:], rhs=gb[:, j, :],
                            start=(j == 0), stop=(j == na - 1),
                        )
                    nc.vector.tensor_copy(out=o1A[:, w, :], in_=ps[:])

            # ---------------- pair barrier (hidden under the A pass) ---------
            barp = mid_es.enter_context(tc.tile_pool(name="bar", bufs=1))
            bar_sb = barp.tile([1, 4], bf16)
            nc.sync.dma_start(out=bar_sb[:], in_=h1tab[0:1, 0:4])
            nc.gpsimd.dma_start(out=bar_in[:, :], in_=bar_sb[:])
            nc.gpsimd.collective_compute(
                "AllReduce", ALU.add,
                ins=[bar_in[:, :]], outs=[bar_out[:, :]], replica_groups=PAIR_RG,
            )
            bar_sb2 = barp.tile([1, 4], bf16)
            nc.sync.dma_start(out=bar_sb2[:], in_=bar_out[:, :])
            # re-zero pad rows from data that depends on the barrier; B-pass
            # gathers read ranges overlapping these rows -> ordered after it
            zdep = barp.tile([N_CORES, D1], bf16)
            nc.vector.memset(zdep[:], 0)
            nc.vector.tensor_scalar_mul(zdep[0:1, 0:4], bar_sb2[:], 0.0)
            nc.gpsimd.dma_start(out=pad_rows_ap(h1tab, D1), in_=zdep[:])

            # ---------------- conv1 B pass: other half + combine + stats -----
            with (
                tc.tile_pool(name="gB", bufs=3) as gpoolB,
                tc.tile_pool(name="sq1", bufs=2) as sqpool,
                tc.tile_pool(name="cBp", bufs=3, space="PSUM") as wpoolB,
                tc.tile_pool(name="st1p", bufs=1, space="PSUM") as stpool,
            ):
                st_s = stpool.tile([1, D1], f32, tag="st_s")
                st_q = stpool.tile([1, D1], f32, tag="st_q")
                for w in range(WPC):
                    nb = int(NB[w])
                    gb = gpoolB.tile([P, NBmax, D1], bf16, tag="gB")
                    nc.gpsimd.dma_gather(
                        gb[:, 0:nb, :], viewB1,
                        idx_s[:, offB[w] // 16: offB[w] // 16 + nb * 8],
                        nb * P, nb * P, D1, elem_step=D1, single_packet=False,
                    )
                    ps = wpoolB.tile([P, D1], f32, tag="winB")
                    for j in range(nb):
                        nc.tensor.matmul(
                            out=ps[:], lhsT=ident_b[:], rhs=gb[:, j, :],
                            start=(j == 0), stop=(j == nb - 1),
                        )
                    nc.vector.tensor_add(o1_all[:, w, :], ps[:], o1A[:, w, :])
                    nc.vector.tensor_scalar_mul(
                        o1_all[:, w, :], o1_all[:, w, :], dismy_s[:, w: w + 1]
                    )
                    sq = sqpool.tile([P, D1], bf16, tag="sq")
                    nc.vector.tensor_mul(sq[:], o1_all[:, w, :], o1_all[:, w, :])
                    mcol = mask_b[:, 1:2] if w == WPC - 1 else mask_b[:, 0:1]
                    nc.tensor.matmul(
                        out=st_s[:], lhsT=mcol, rhs=o1_all[:, w, :],
                        start=(w == 0), stop=(w == WPC - 1), skip_group_check=True,
                    )
                    nc.tensor.matmul(
                        out=st_q[:], lhsT=mcol, rhs=sq[:],
                        start=(w == 0), stop=(w == WPC - 1), skip_group_check=True,
                    )
                stats1 = o1_pool.tile([1, 2 * D1], f32)
                nc.vector.tensor_copy(out=stats1[:, 0:D1], in_=st_s[:])
                nc.vector.tensor_copy(out=stats1[:, D1:], in_=st_q[:])
            if phases < 3:
                outst = cpool.tile([P, WPC, D2], f32)
                nc.vector.tensor_copy(out=outst[:], in_=o1_all[:, :, 0:D2])
                nc.gpsimd.dma_start(
                    out=out_d[0:SLOTS, :].rearrange("(t p) d -> p t d", p=P),
                    in_=outst[:],
                )
                mid_es.close()
                return nc
            nc.gpsimd.dma_start(out=ar1_in[:, :], in_=stats1[:])
            nc.gpsimd.collective_compute(
                "AllReduce", ALU.add,
                ins=[ar1_in[:, :]], outs=[ar1_out[:, :]], replica_groups=RG,
            )

            # ---------------- BN1 factors + h2 shard ----------------
            bnp = mid_es.enter_context(tc.tile_pool(name="bn1", bufs=1))
            sg = bnp.tile([1, 2 * D1], f32)
            nc.sync.dma_start(out=sg[:], in_=ar1_out[:, :])
            mean = bnp.tile([1, D1], f32)
            nc.vector.tensor_scalar_mul(mean[:], sg[:, 0:D1], 1.0 / N)
            ex2 = bnp.tile([1, D1], f32)
            nc.vector.tensor_scalar_mul(ex2[:], sg[:, D1:], 1.0 / N)
            var = bnp.tile([1, D1], f32)
            nc.vector.tensor_mul(var[:], mean[:], mean[:])
            nc.vector.tensor_sub(var[:], ex2[:], var[:])
            epst = bnp.tile([1, 1], f32)
            nc.vector.memset(epst[:], EPS)
            sd = bnp.tile([1, D1], f32)
            nc.scalar.activation(sd[:], var[:], AF.Sqrt, bias=epst[:])
            rstd = bnp.tile([1, D1], f32)
            nc.vector.reciprocal(rstd[:], sd[:])
            a1 = bnp.tile([1, D1], f32)
            nc.vector.tensor_mul(a1[:], rstd[:], gb_s[:, 0:D1])
            c1 = bnp.tile([1, D1], f32)
            nc.vector.tensor_mul(c1[:], mean[:], a1[:])
            nc.vector.tensor_sub(c1[:], gb_s[:, D1: 2 * D1], c1[:])
            # transpose (a1, c1) -> per-partition chunks [128, 2] per KC2 chunk
            acT = bnp.tile([P, KC2, 2], f32)
            with tc.tile_pool(name="trp", bufs=4, space="PSUM") as trpool:
                for c in range(KC2):
                    tpa = trpool.tile([P, 1], f32, tag="tra")
                    nc.tensor.transpose(
                        out=tpa[:], in_=a1[:, c * P: (c + 1) * P],
                        identity=ident_f[0:1, 0:1],
                    )
                    nc.vector.tensor_copy(out=acT[:, c, 0:1], in_=tpa[:])
                    tpc = trpool.tile([P, 1], f32, tag="trc")
                    nc.tensor.transpose(
                        out=tpc[:], in_=c1[:, c * P: (c + 1) * P],
                        identity=ident_f[0:1, 0:1],
                    )
                    nc.vector.tensor_copy(out=acT[:, c, 1:2], in_=tpc[:])

            # per window: transpose o1 chunk, BN+ReLU (split ACT/DVE), W2 matmul
            h2stage = bnp.tile([P, WPC, D2], bf16)
            with (
                tc.tile_pool(name="bnr", bufs=4) as bpool,
                tc.tile_pool(name="h2p", bufs=2, space="PSUM") as h2pool,
                tc.tile_pool(name="trq", bufs=4, space="PSUM") as trq,
            ):
                for w in range(WPC):
                    h2ps = h2pool.tile([P, D2], f32, tag="h2ps")
                    for c in range(KC2):
                        tp = trq.tile([P, P], bf16, tag="tr")
                        nc.tensor.transpose(
                            out=tp[:], in_=o1_all[:, w, c * P: (c + 1) * P],
                            identity=ident_b[:],
                        )
                        bnr = bpool.tile([P, P], bf16, tag="bnr")
                        if w % 2 == 0:
                            nc.scalar.activation(
                                bnr[:], tp[:], AF.Relu,
                                bias=acT[:, c, 1:2], scale=acT[:, c, 0:1],
                            )
                        else:
                            nc.vector.tensor_scalar(
                                out=bnr[:], in0=tp[:],
                                scalar1=acT[:, c, 0:1], scalar2=acT[:, c, 1:2],
                                op0=ALU.mult, op1=ALU.add,
                            )
                            nc.vector.tensor_scalar_max(bnr[:], bnr[:], 0.0)
                        nc.tensor.matmul(
                            out=h2ps[:], lhsT=bnr[:], rhs=w2_s[:, c, :],
                            start=(c == 0), stop=(c == KC2 - 1),
                        )
                    nc.vector.tensor_scalar_mul(
                        h2stage[:, w, :], h2ps[:], dismy_s[:, w: w + 1]
                    )
            zrow2 = bnp.tile([1, D2], bf16)
            nc.vector.memset(zrow2[:], 0)
            nc.gpsimd.dma_start(
                out=h2shard[0:SLOTS, :].rearrange("(t p) d -> p t d", p=P),
                in_=h2stage[:],
            )
            nc.gpsimd.dma_start(out=h2shard[SLOTS:BLK, :], in_=zrow2[:])
            nc.gpsimd.collective_compute(
                "AllGather", ALU.bypass,
                ins=[h2shard[:, :]], outs=[h2tab[:, :]], replica_groups=RG,
            )
            if phases < 4:
                outst = cpool.tile([P, WPC, D2], f32)
                nc.vector.tensor_copy(out=outst[:], in_=h2stage[:])
                nc.gpsimd.dma_start(
                    out=out_d[0:SLOTS, :].rearrange("(t p) d -> p t d", p=P),
                    in_=outst[:],
                )
                mid_es.close()
                return nc
            mid_es.close()

            # ---------------- conv2 (A+B into one PSUM per window) -----------
            o2_pool = es.enter_context(tc.tile_pool(name="o2", bufs=1))
            o2_all = o2_pool.tile([P, WPC, D2], f32)
            viewA2 = h2tab[ds(parity * HB, HB), :]
            viewB2 = h2tab[ds(HB - parity * HB, HB), :]
            with (
                tc.tile_pool(name="g2", bufs=3) as gpool2,
                tc.tile_pool(name="sq2", bufs=2) as sqpool2,
                tc.tile_pool(name="c2p", bufs=3, space="PSUM") as wpool2,
                tc.tile_pool(name="st2p", bufs=1, space="PSUM") as stpool2,
            ):
                st2_s = stpool2.tile([1, D2], f32, tag="st2_s")
                st2_q = stpool2.tile([1, D2], f32, tag="st2_q")
                for w in range(WPC):
                    na, nb = int(NA[w]), int(NB[w])
                    nt = na + nb
                    gb = gpool2.tile([P, NAmax + NBmax, D2], bf16, tag="g2")
                    nc.gpsimd.dma_gather(
                        gb[:, 0:na, :], viewA2,
                        idx_s[:, offA[w] // 16: offA[w] // 16 + na * 8],
                        na * P, na * P, D2, elem_step=D2, single_packet=False,
                    )
                    nc.gpsimd.dma_gather(
                        gb[:, na:nt, :], viewB2,
                        idx_s[:, offB[w] // 16: offB[w] // 16 + nb * 8],
                        nb * P, nb * P, D2, elem_step=D2, single_packet=False,
                    )
                    ps = wpool2.tile([P, D2], f32, tag="win2")
                    for j in range(nt):
                        nc.tensor.matmul(
                            out=ps[:], lhsT=ident_b[:], rhs=gb[:, j, :],
                            start=(j == 0), stop=(j == nt - 1),
                        )
                    nc.vector.tensor_scalar_mul(
                        o2_all[:, w, :], ps[:], dismy_s[:, w: w + 1]
                    )
                    o2b = sqpool2.tile([P, D2], bf16, tag="o2b")
                    nc.vector.tensor_copy(out=o2b[:], in_=o2_all[:, w, :])
                    sq = sqpool2.tile([P, D2], bf16, tag="sq2")
                    nc.vector.tensor_mul(sq[:], o2_all[:, w, :], o2_all[:, w, :])
                    mcol = mask_b[:, 1:2] if w == WPC - 1 else mask_b[:, 0:1]
                    nc.tensor.matmul(
                        out=st2_s[:], lhsT=mcol, rhs=o2b[:],
                        start=(w == 0), stop=(w == WPC - 1), skip_group_check=True,
                    )
                    nc.tensor.matmul(
                        out=st2_q[:], lhsT=mcol, rhs=sq[:],
                        start=(w == 0), stop=(w == WPC - 1), skip_group_check=True,
                    )
                stats2 = o2_pool.tile([1, 2 * D2], f32)
                nc.vector.tensor_copy(out=stats2[:, 0:D2], in_=st2_s[:])
                nc.vector.tensor_copy(out=stats2[:, D2:], in_=st2_q[:])
            if phases < 5:
                outst = cpool.tile([P, WPC, D2], f32)
                nc.vector.tensor_copy(out=outst[:], in_=o2_all[:, :, :])
                nc.gpsimd.dma_start(
                    out=out_d[0:SLOTS, :].rearrange("(t p) d -> p t d", p=P),
                    in_=outst[:],
                )
                return nc
            nc.gpsimd.dma_start(out=ar2_in[:, :], in_=stats2[:])
            nc.gpsimd.collective_compute(
                "AllReduce", ALU.add,
                ins=[ar2_in[:, :]], outs=[ar2_out[:, :]], replica_groups=RG,
            )

            # ---------------- BN2 + output ----------------
            sg2 = o2_pool.tile([1, 2 * D2], f32)
            nc.sync.dma_start(out=sg2[:], in_=ar2_out[:, :])
            mean2 = o2_pool.tile([1, D2], f32)
            nc.vector.tensor_scalar_mul(mean2[:], sg2[:, 0:D2], 1.0 / N)
            ex22 = o2_pool.tile([1, D2], f32)
            nc.vector.tensor_scalar_mul(ex22[:], sg2[:, D2:], 1.0 / N)
            var2 = o2_pool.tile([1, D2], f32)
            nc.vector.tensor_mul(var2[:], mean2[:], mean2[:])
            nc.vector.tensor_sub(var2[:], ex22[:], var2[:])
            epst2 = o2_pool.tile([1, 1], f32)
            nc.vector.memset(epst2[:], EPS)
            sd2 = o2_pool.tile([1, D2], f32)
            nc.scalar.activation(sd2[:], var2[:], AF.Sqrt, bias=epst2[:])
            rstd2 = o2_pool.tile([1, D2], f32)
            nc.vector.reciprocal(rstd2[:], sd2[:])
            a2 = o2_pool.tile([1, D2], f32)
            nc.vector.tensor_mul(a2[:], rstd2[:], gb_s[:, 2 * D1: 2 * D1 + D2])
            c2 = o2_pool.tile([1, D2], f32)
            nc.vector.tensor_mul(c2[:], mean2[:], a2[:])
            nc.vector.tensor_sub(c2[:], gb_s[:, 2 * D1 + D2:], c2[:])

            # broadcast a2/c2 across partitions via ones-column matmul
            onesrow = o2_pool.tile([1, P], f32)
            nc.vector.memset(onesrow[:], 1.0)
            a2b = o2_pool.tile([P, D2], f32)
            c2b = o2_pool.tile([P, D2], f32)
            with tc.tile_pool(name="bn2p", bufs=2, space="PSUM") as bn2p:
                bps = bn2p.tile([P, D2], f32, tag="b2a")
                nc.tensor.matmul(out=bps[:], lhsT=onesrow[:], rhs=a2[:], start=True, stop=True)
                nc.vector.tensor_copy(out=a2b[:], in_=bps[:])
                cps = bn2p.tile([P, D2], f32, tag="b2c")
                nc.tensor.matmul(out=cps[:], lhsT=onesrow[:], rhs=c2[:], start=True, stop=True)
                nc.vector.tensor_copy(out=c2b[:], in_=cps[:])

            outst = o2_pool.tile([P, WPC, D2], f32)
            for w in range(WPC):
                nc.vector.tensor_mul(outst[:, w, :], o2_all[:, w, :], a2b[:])
                nc.vector.tensor_add(outst[:, w, :], outst[:, w, :], c2b[:])
            nc.gpsimd.dma_start(
                out=out_d[0:SLOTS, :].rearrange("(t p) d -> p t d", p=P),
                in_=outst[:],
            )

    return nc


# ---------------------------------------------------------------- entry point

def _run(x, edge_index, W1, gamma1, beta1, W2, gamma2, beta2, cfg, trace=False):
    from concourse.bass_utils import run_bass_kernel_spmd

    N = cfg["N"]
    pp = _preprocess(edge_index, N)
    xb, w1b, w2b = _pack_inputs(np.asarray(x, np.float32), np.asarray(W1, np.float32),
                                np.asarray(W2, np.float32), pp, cfg)
    nc = _build_kernel(cfg, pp, phases=int(__import__("os").environ.get("K_PHASES", "5")))
    nc.compile()

    NT2 = pp["NTILES"] // 2
    shared = {
        "w1b": np.ascontiguousarray(w1b),
        "w2b": np.ascontiguousarray(w2b),
        "statmask": np.ascontiguousarray(pp["statmask"]),
        "gamma1": np.asarray(gamma1, np.float32).reshape(1, -1),
        "beta1": np.asarray(beta1, np.float32).reshape(1, -1),
        "gamma2": np.asarray(gamma2, np.float32).reshape(1, -1),
        "beta2": np.asarray(beta2, np.float32).reshape(1, -1),
    }
    xb_lo = np.ascontiguousarray(xb[:NT2])
    xb_hi = np.ascontiguousarray(xb[NT2:])
    in_maps = []
    for c in range(N_CORES):
        m = dict(shared)
        m["xb"] = xb_lo if c % 2 == 0 else xb_hi
        m["idx"] = np.ascontiguousarray(pp["idx_wrapped"][c])
        m["dismy"] = np.ascontiguousarray(pp["dismy"][c])
        in_maps.append(m)

    res = run_bass_kernel_spmd(nc, in_maps, core_ids=list(range(N_CORES)), trace=trace)
    _run.last_nc = nc

    D2 = cfg["D2"]
    out = np.empty((N, D2), np.float32)
    pos, core_of = pp["pos"], pp["core_of"]
    for c in range(N_CORES):
        nodes = np.flatnonzero(core_of == c)
        out[nodes] = res.results[c]["out"][pos[nodes]]
    _run.last_result = res
    return out


def kernel(x, edge_index, W1, b1, gamma1, beta1, W2, b2, gamma2, beta2):
    # b1/b2 cancel exactly through BatchNorm's mean subtraction; unused.
    return _run(x, edge_index, W1, gamma1, beta1, W2, gamma2, beta2, _FULL_CFG)


# revision 9
# speedup vs baseline: 1.0256x; 1.0256x over previous
"""GCN encoder (2x GCNConv + BatchNorm + ReLU) on 8 Trainium2 NeuronCores.

Strategy (graph/data parallel, per sharding hint):
- Nodes are sharded across the 8 cores; each core owns 49 "windows" of 128
  destination nodes.  Source nodes are split into a "lo" half (owned by cores
  0-3) and "hi" half (cores 4-7); the half assignment is optimized on the host
  (discrepancy balancing) so each destination's in-edges split ~evenly, which
  makes the per-window gather-chunk maxima tight (low padding waste).
- norm factorizes: norm(s,d) = dis[s]*dis[d].  Source scaling dis[s] is folded
  into the feature tables (h~ = dis * h); destination scaling dis[d] is applied
  on eviction.  Messages aggregate with a constant identity matmul into PSUM.
- Gathers use the int16 dma_gather embedding path; each window has an "A"
  segment (sources in the half this core's HBM-pair parity built) and a "B"
  segment (other half); pad slots point at an all-zero row.
- Phase 1 (h1 = dis*(x @ W1)) is split across HBM-pair cores: the pair shares
  one h1 table (addr_space="Shared"); the even core computes/writes the lo
  half, the odd core the hi half (rank-dependent write offsets via
  partition_id + DynSlice).  A 2-core AllReduce barrier syncs the pair; it is
  hidden behind conv1's "A" pass, which only reads the self-built half.
- h2 = relu(bn1(conv1)) @ W2 is computed per-shard; an AllGather replicates
  the h2 table for conv2.  BatchNorm stats use E[x^2]-mean^2 via ones-vector
  matmuls accumulated in PSUM, then AllReduce.  b1/b2 cancel in BN.
"""

import sys

sys.path.insert(0, "/opt/trn_rl_repo")

import numpy as np

N_CORES = 8
P = 128
EPS = 1e-5

_FULL_CFG = dict(N=50000, IN=512, D1=256, D2=128)


# ---------------------------------------------------------------- host preprocessing

def _balance_halves(S, D, deg, N, seed=0):
    """Assign each node to the lo (+1) or hi (-1) half so that every dst's
    in-edges split ~evenly between halves.  Greedy discrepancy minimization."""
    order_s = np.argsort(S, kind="stable")
    Ds = D[order_s]
    starts = np.searchsorted(S[order_s], np.arange(N + 1))
    outdeg = np.diff(starts)
    rng = np.random.default_rng(seed)
    h = np.where(rng.random(N) < 0.5, 1, -1)

    for r in range(120):
        cur = np.bincount(D, weights=h[S].astype(np.float64), minlength=N)
        s_cursum = np.add.reduceat(cur[Ds], starts[:-1]) if len(Ds) else np.zeros(N)
        s_cursum[outdeg == 0] = 0
        gain = h * s_cursum - outdeg
        batch = 3000 if r < 10 else (800 if r < 40 else 250)
        lo_c = np.flatnonzero((gain > 0) & (h == 1))
        hi_c = np.flatnonzero((gain > 0) & (h == -1))
        nlo = int((h == 1).sum())
        k_lo = min(len(lo_c), batch + max(0, nlo - N // 2))
        k_hi = min(len(hi_c), batch + max(0, N // 2 - nlo))
        if k_lo + k_hi == 0:
            break
        h[lo_c[np.argsort(-gain[lo_c])[:k_lo]]] = -1
        h[hi_c[np.argsort(-gain[hi_c])[:k_hi]]] = 1

    cur = np.bincount(D, weights=h[S].astype(np.float64), minlength=N).astype(np.int64)
    in_order = np.argsort(D, kind="stable")
    Sin = S[in_order]
    in_starts = np.searchsorted(D[in_order], np.arange(N + 1))
    for sweep in range(6):
        bad = np.flatnonzero(np.abs(cur) >= 3)
        bad = bad[np.argsort(-np.abs(cur[bad]))]
        if len(bad) == 0:
            break
        for d in bad:
            cd = cur[d]
            if abs(cd) < 3:
                continue
            sign = 1 if cd > 0 else -1
            nbrs = Sin[in_starts[d]:in_starts[d + 1]]
            cands = nbrs[h[nbrs] == sign]
            if len(cands) == 0:
                continue
            best, bestg = -1, -(10 ** 9)
            for s in cands[:12]:
                od = Ds[starts[s]:starts[s + 1]]
                g = h[s] * cur[od].sum() - len(od)
                if g > bestg:
                    best, bestg = s, g
            od = Ds[starts[best]:starts[best + 1]]
            cur[od] -= 2 * h[best]
            h[best] = -h[best]
    # exact 50/50 split
    nlo = int((h == 1).sum())
    if nlo != N // 2:
        d = 1 if nlo > N // 2 else -1
        side = np.flatnonzero(h == d)
        s_cursum = np.add.reduceat(cur[Ds], starts[:-1]) if len(Ds) else np.zeros(N)
        s_cursum[outdeg == 0] = 0
        gain = h * s_cursum - outdeg
        take = side[np.argsort(-gain[side])[: abs(nlo - N // 2)]]
        for s in take:
            od = Ds[starts[s]:starts[s + 1]]
            cur[od] -= 2 * h[s]
            h[s] = -h[s]
    return h


def _preprocess(edge_index, N):
    """Graph preprocessing: half balancing, node permutation, A/B segment
    chunk assignment, gather indices.  Pure integer work on the host."""
    src = np.asarray(edge_index[0], dtype=np.int64)
    dst = np.asarray(edge_index[1], dtype=np.int64)
    loop = np.arange(N, dtype=np.int64)
    S = np.concatenate([src, loop])
    D = np.concatenate([dst, loop])

    deg = np.bincount(D, minlength=N)  # >= 1 (self loop)
    dis = (1.0 / np.sqrt(deg.astype(np.float64))).astype(np.float32)

    real_pc = N // N_CORES
    WPC = (real_pc + P - 1) // P          # windows per core
    SLOTS = WPC * P                        # slot positions per core
    BLK = SLOTS + 1                        # +1 trailing zero row per core block

    # half assignment (lo = cores 0-3), then deal each half by degree
    hsplit = _balance_halves(S, D, deg, N)
    core_of = np.empty(N, dtype=np.int64)
    for half, hv in ((0, 1), (1, -1)):
        nodes = np.flatnonzero(hsplit == hv)
        o = nodes[np.argsort(-deg[nodes], kind="stable")]
        core_of[o] = half * 4 + np.arange(len(o)) % 4

    half_node = core_of >= (N_CORES // 2)  # True = hi half
    halfE = half_node[S]
    deg_lo = np.bincount(D[~halfE], minlength=N)
    deg_hi = deg - deg_lo

    # position within core: sort by max(lo,hi) desc (then total) — with the
    # balanced halves lo≈hi, this keeps BOTH per-window maxima tight
    pos = np.empty(N, dtype=np.int64)
    node_by_cp = np.full((N_CORES, SLOTS), -1, dtype=np.int64)
    for c in range(N_CORES):
        nodes_c = np.flatnonzero(core_of == c)
        key = np.maximum(deg_lo[nodes_c], deg_hi[nodes_c])
        o = np.lexsort((-(deg_lo[nodes_c] + deg_hi[nodes_c]), -key))
        snodes = nodes_c[o]
        pos[snodes] = np.arange(len(snodes))
        node_by_cp[c, : len(snodes)] = snodes

    # per-core per-window maxima of lo/hi degree
    dlo_cp = np.zeros((N_CORES, SLOTS), dtype=np.int64)
    dhi_cp = np.zeros((N_CORES, SLOTS), dtype=np.int64)
    m = node_by_cp >= 0
    dlo_cp[m] = deg_lo[node_by_cp[m]]
    dhi_cp[m] = deg_hi[node_by_cp[m]]
    WL = dlo_cp.reshape(N_CORES, WPC, P).max(axis=2)  # [core, w]
    WH = dhi_cp.reshape(N_CORES, WPC, P).max(axis=2)
    ev = np.arange(N_CORES) % 2 == 0
    # segment A = the half this core's pair-parity built (even: lo, odd: hi)
    NA = np.maximum(WL[ev].max(axis=0), WH[~ev].max(axis=0))
    NB = np.maximum(WH[ev].max(axis=0), WL[~ev].max(axis=0))

    # idx segment offsets: per window [A seg][B seg], chunk-major inside
    seg = (NA + NB) * P
    base = np.concatenate([[0], np.cumsum(seg)])
    offA = base[:-1]
    offB = base[:-1] + NA * P
    TOT = int(base[-1])

    # edge -> segment: A iff src half == dst-core's parity half
    cD = core_of[D]
    inA = halfE == (cD % 2 == 1)
    key = D * 2 + (~inA).astype(np.int64)
    ksort = np.argsort(key, kind="stable")
    skey = key[ksort]
    starts = np.concatenate([[0], np.flatnonzero(np.diff(skey)) + 1])
    group_len = np.diff(np.concatenate([starts, [len(skey)]]))
    chunk_sorted = np.arange(len(skey)) - np.repeat(starts, group_len)
    chunk = np.empty(len(S), dtype=np.int64)
    chunk[ksort] = chunk_sorted

    wD = pos[D] // P
    slotD = pos[D] % P
    rel = (core_of[S] % (N_CORES // 2)) * BLK + pos[S]  # within-half row
    assert rel.max() < 32768
    epos = np.where(inA, offA[wD], offB[wD]) + chunk * P + slotD

    PADIDX = SLOTS  # block 0's trailing zero row (within-half view)
    flat = np.full(N_CORES * TOT, PADIDX, dtype=np.int16)
    flat[cD * TOT + epos] = rel.astype(np.int16)
    flat = flat.reshape(N_CORES, TOT)
    # wrap: idx i -> [i%16, i//16], replicated across the 8 groups of 16 rows
    wrapped16 = flat.reshape(N_CORES, TOT // 16, 16).transpose(0, 2, 1)
    idx_wrapped = np.tile(wrapped16, (1, P // 16, 1))  # [cores, 128, TOT/16]

    # per-core dis (by slot), 1.0 for dummies
    dis_cp = np.ones((N_CORES, SLOTS), dtype=np.float32)
    dis_cp[m] = dis[node_by_cp[m]]
    dismy = dis_cp.reshape(N_CORES, WPC, P).transpose(0, 2, 1)  # [c, 128, WPC]

    NTILES = N_CORES * WPC

    # stats mask: last window has (SLOTS - real_pc) dummy rows at the end
    n_dummy = SLOTS - real_pc
    statmask = np.ones((P, 2), dtype=np.float32)
    if n_dummy:
        statmask[P - n_dummy:, 1] = 0.0

    waste = float(seg.sum()) / max(1, len(S) / N_CORES) - 1.0
    return dict(
        WPC=WPC, SLOTS=SLOTS, BLK=BLK, NTILES=NTILES,
        NA=NA.astype(int), NB=NB.astype(int), TOT=TOT,
        offA=offA, offB=offB,
        idx_wrapped=idx_wrapped, dismy=dismy,
        statmask=statmask, node_by_cp=node_by_cp, pos=pos, core_of=core_of,
        dis=dis, real_pc=real_pc, waste=waste,
    )


def _pack_inputs(x, W1, W2, pp, cfg):
    """Build the device input arrays.  xb is the full permuted/prescaled x in
    table order; each core receives only the half it builds in phase 1."""
    import ml_dtypes

    bf16 = ml_dtypes.bfloat16
    N, IN, D1, D2 = cfg["N"], cfg["IN"], cfg["D1"], cfg["D2"]
    SLOTS, NTILES = pp["SLOTS"], pp["NTILES"]
    KC = IN // P

    xperm = np.zeros((N_CORES * SLOTS, IN), dtype=np.float32)
    m = pp["node_by_cp"] >= 0
    xperm[np.flatnonzero(m.reshape(-1))] = (
        x[pp["node_by_cp"][m]] * pp["dis"][pp["node_by_cp"][m]][:, None]
    )
    xb = (
        xperm.reshape(NTILES, P, KC, P)   # [b, j, kc, p]
        .transpose(0, 3, 2, 1)            # [b, p, kc, j]
        .astype(bf16)
    )
    w1b = W1.reshape(KC, P, D1).transpose(1, 0, 2).astype(bf16)   # [p, kc, D1]
    w2b = W2.reshape(D1 // P, P, D2).transpose(1, 0, 2).astype(bf16)  # [p, kc, D2]
    return xb, w1b, w2b


# ---------------------------------------------------------------- device kernel

def _build_kernel(cfg, pp, phases=5):
    import concourse.bacc as bacc
    import concourse.mybir as mybir
    import concourse.tile as tile
    from concourse.masks import make_identity
    from concourse.bass import ds
    from contextlib import ExitStack

    N, IN, D1, D2 = cfg["N"], cfg["IN"], cfg["D1"], cfg["D2"]
    WPC, SLOTS, BLK, NTILES = pp["WPC"], pp["SLOTS"], pp["BLK"], pp["NTILES"]
    NA, NB, TOT = pp["NA"], pp["NB"], pp["TOT"]
    offA, offB = pp["offA"], pp["offB"]
    KC = IN // P
    KC2 = D1 // P
    HB = (N_CORES // 2) * BLK        # rows per half
    NROWS = N_CORES * BLK            # table rows
    NBLK = N_CORES // 2              # blocks built per core (its parity half)
    NTILES2 = NBLK * WPC             # xb tiles per core
    NAmax = int(NA.max())
    NBmax = int(NB.max())
    RG = [list(range(N_CORES))]
    PAIR_RG = [[2 * k, 2 * k + 1] for k in range(N_CORES // 2)]
    f32, bf16, i16 = mybir.dt.float32, mybir.dt.bfloat16, mybir.dt.int16
    AF = mybir.ActivationFunctionType
    ALU = mybir.AluOpType

    nc = bacc.Bacc(num_devices=N_CORES)

    # ---- I/O
    xb_d = nc.dram_tensor("xb", [NTILES2, P, KC, P], bf16, kind="ExternalInput")
    w1_d = nc.dram_tensor("w1b", [P, KC, D1], bf16, kind="ExternalInput")
    w2_d = nc.dram_tensor("w2b", [P, KC2, D2], bf16, kind="ExternalInput")
    idx_d = nc.dram_tensor("idx", [P, TOT // 16], i16, kind="ExternalInput")
    dismy_d = nc.dram_tensor("dismy", [P, WPC], f32, kind="ExternalInput")
    mask_d = nc.dram_tensor("statmask", [P, 2], f32, kind="ExternalInput")
    g1_d = nc.dram_tensor("gamma1", [1, D1], f32, kind="ExternalInput")
    b1_d = nc.dram_tensor("beta1", [1, D1], f32, kind="ExternalInput")
    g2_d = nc.dram_tensor("gamma2", [1, D2], f32, kind="ExternalInput")
    b2_d = nc.dram_tensor("beta2", [1, D2], f32, kind="ExternalInput")
    out_d = nc.dram_tensor("out", [SLOTS, D2], f32, kind="ExternalOutput")

    # ---- internal DRAM
    # h1tab is shared within an HBM core pair: the even core writes the lo
    # half, the odd core the hi half.
    h1tab = nc.dram_tensor("h1tab", [NROWS, D1], bf16, kind="Internal",
                           addr_space="Shared")
    h2shard = nc.dram_tensor("h2shard", [BLK, D2], bf16, kind="Internal")
    h2tab = nc.dram_tensor("h2tab", [NROWS, D2], bf16, kind="Internal", addr_space="Shared")
    bar_in = nc.dram_tensor("bar_in", [1, 4], bf16, kind="Internal")
    bar_out = nc.dram_tensor("bar_out", [1, 4], bf16, kind="Internal")
    ar1_in = nc.dram_tensor("ar1_in", [1, 2 * D1], f32, kind="Internal")
    ar1_out = nc.dram_tensor("ar1_out", [1, 2 * D1], f32, kind="Internal", addr_space="Shared")
    ar2_in = nc.dram_tensor("ar2_in", [1, 2 * D2], f32, kind="Internal")
    ar2_out = nc.dram_tensor("ar2_out", [1, 2 * D2], f32, kind="Internal", addr_space="Shared")

    import concourse.bass as bass

    def pad_rows_ap(tensor, Dd):
        # rows {c*BLK + SLOTS : c in 0..7} of a [NROWS, Dd] table
        return bass.AP(tensor, SLOTS * Dd, [[BLK * Dd, N_CORES], [1, Dd]])

    with tile.TileContext(nc) as tc:
        es = ExitStack()
        with es:
            parity = nc.gpsimd.partition_id() % 2

            cpool = es.enter_context(tc.tile_pool(name="const", bufs=1))
            ident_b = cpool.tile([P, P], bf16)
            make_identity(nc, ident_b[:])
            ident_f = cpool.tile([P, P], f32)
            make_identity(nc, ident_f[:])
            w1_s = cpool.tile([P, KC, D1], bf16)
            nc.sync.dma_start(out=w1_s[:], in_=w1_d[:, :, :])
            w2_s = cpool.tile([P, KC2, D2], bf16)
            nc.sync.dma_start(out=w2_s[:], in_=w2_d[:, :, :])
            dismy_s = cpool.tile([P, WPC], f32)
            nc.sync.dma_start(out=dismy_s[:], in_=dismy_d[:, :])
            mask_s = cpool.tile([P, 2], f32)
            nc.sync.dma_start(out=mask_s[:], in_=mask_d[:, :])
            mask_b = cpool.tile([P, 2], bf16)
            nc.vector.tensor_copy(out=mask_b[:], in_=mask_s[:])
            idx_s = cpool.tile([P, TOT // 16], i16)
            nc.sync.dma_start(out=idx_s[:], in_=idx_d[:, :])
            gb_s = cpool.tile([1, 2 * D1 + 2 * D2], f32)  # gamma1|beta1|gamma2|beta2
            nc.sync.dma_start(out=gb_s[:, 0:D1], in_=g1_d[:, :])
            nc.sync.dma_start(out=gb_s[:, D1: 2 * D1], in_=b1_d[:, :])
            nc.sync.dma_start(out=gb_s[:, 2 * D1: 2 * D1 + D2], in_=g2_d[:, :])
            nc.sync.dma_start(out=gb_s[:, 2 * D1 + D2:], in_=b2_d[:, :])

            # zero ALL pad rows (both pair cores write identical zeros: benign)
            zrow = cpool.tile([N_CORES, D1], bf16)
            nc.vector.memset(zrow[:], 0)
            nc.gpsimd.dma_start(out=pad_rows_ap(h1tab, D1), in_=zrow[:])

            # ---------------- phase 1: my parity half of h1tab ----------------
            with (
                tc.tile_pool(name="p1x", bufs=6) as xpool,
                tc.tile_pool(name="p1s", bufs=2) as spool,
                tc.tile_pool(name="p1p", bufs=4, space="PSUM") as ppool1,
            ):
                XB = 7 if WPC % 7 == 0 else 1   # x tiles per DMA
                for blk in range(NBLK):
                    stage = spool.tile([P, WPC, D1], bf16, tag="stage")
                    for tb in range(WPC // XB):
                        b0 = blk * WPC + tb * XB
                        xt = xpool.tile([P, XB, KC, P], bf16, tag="xt")
                        nc.sync.dma_start(
                            out=xt[:], in_=xb_d[b0: b0 + XB].rearrange("b p k j -> p b k j")
                        )
                        for t2 in range(XB):
                            t = tb * XB + t2
                            ps = ppool1.tile([P, D1], f32, tag="ps1")
                            for kc in range(KC):
                                nc.tensor.matmul(
                                    out=ps[:], lhsT=xt[:, t2, kc, :], rhs=w1_s[:, kc, :],
                                    start=(kc == 0), stop=(kc == KC - 1),
                                )
                            nc.scalar.activation(
                                stage[:, t, :], ps[:], AF.Copy
                            )
                    nc.gpsimd.dma_start(
                        out=h1tab[ds(parity * HB + blk * BLK, SLOTS), :].rearrange(
                            "(t p) d -> p t d", p=P
                        ),
                        in_=stage[:],
                    )

            if phases < 2:
                outst = cpool.tile([P, WPC, D2], f32)
                nc.vector.memset(outst[:], 0)
                nc.gpsimd.dma_start(
                    out=out_d[0:SLOTS, :].rearrange("(t p) d -> p t d", p=P),
                    in_=outst[:],
                )
                return nc

            # ---------------- pair barrier (issued early, hidden under A) ----
            mid_es = ExitStack()
            o1_pool = mid_es.enter_context(tc.tile_pool(name="o1", bufs=1))
            barp = mid_es.enter_context(tc.tile_pool(name="bar", bufs=1))
            bar_sb = barp.tile([1, 4], bf16)
            nc.sync.dma_start(out=bar_sb[:], in_=h1tab[0:1, 0:4])
            nc.gpsimd.dma_start(out=bar_in[:, :], in_=bar_sb[:])
            nc.gpsimd.collective_compute(
                "AllReduce", ALU.add,
                ins=[bar_in[:, :]], outs=[bar_out[:, :]], replica_groups=PAIR_RG,
            )

            # ---------------- conv1 A pass: my-half gathers (no barrier) -----
            o1A = o1_pool.tile([P, WPC, D1], bf16)     # A partials
            o1_all = o1_pool.tile([P, WPC, D1], bf16)  # combined conv1 out
            viewA1 = h1tab[ds(parity * HB, HB), :]
            viewB1 = h1tab[ds(HB - parity * HB, HB), :]
            with (
                tc.tile_pool(name="gA", bufs=3) as gpoolA,
                tc.tile_pool(name="cAp", bufs=3, space="PSUM") as wpoolA,
            ):
                for w in range(WPC):
                    na = int(NA[w])
                    gb = gpoolA.tile([P, NAmax, D1], bf16, tag="gA")
                    nc.gpsimd.dma_gather(
                        gb[:, 0:na, :], viewA1,
                        idx_s[:, offA[w] // 16: offA[w] // 16 + na * 8],
                        na * P, na * P, D1, elem_step=D1, single_packet=False,
                    )
                    ps = wpoolA.tile([P, D1], f32, tag="winA")
                    for j in range(na):
                        nc.tensor.matmul(
                            out=ps[:], lhsT=ident_b[:], rhs=gb[:, j, :],
                            start=(j == 0), stop=(j == na - 1),
                        )
                    nc.vector.tensor_copy(out=o1A[:, w, :], in_=ps[:])

            # barrier completion gate: re-zero pad rows from data that depends
            # on the collective output; B-pass gathers read ranges overlapping
            # these rows -> ordered after it
            bar_sb2 = barp.tile([1, 4], bf16)
            nc.sync.dma_start(out=bar_sb2[:], in_=bar_out[:, :])
            zdep = barp.tile([N_CORES, D1], bf16)
            nc.vector.memset(zdep[:], 0)
            nc.vector.tensor_scalar_mul(zdep[0:1, 0:4], bar_sb2[:], 0.0)
            nc.gpsimd.dma_start(out=pad_rows_ap(h1tab, D1), in_=zdep[:])

            # ---------------- conv1 B pass: other half + combine + stats -----
            with (
                tc.tile_pool(name="gB", bufs=3) as gpoolB,
                tc.tile_pool(name="sq1", bufs=2) as sqpool,
                tc.tile_pool(name="cBp", bufs=3, space="PSUM") as wpoolB,
                tc.tile_pool(name="st1p", bufs=1, space="PSUM") as stpool,
            ):
                st_s = stpool.tile([1, D1], f32, tag="st_s")
                st_q = stpool.tile([1, D1], f32, tag="st_q")
                for w in range(WPC):
                    nb = int(NB[w])
                    gb = gpoolB.tile([P, NBmax, D1], bf16, tag="gB")
                    nc.gpsimd.dma_gather(
                        gb[:, 0:nb, :], viewB1,
                        idx_s[:, offB[w] // 16: offB[w] // 16 + nb * 8],
                        nb * P, nb * P, D1, elem_step=D1, single_packet=False,
                    )
                    ps = wpoolB.tile([P, D1], f32, tag="winB")
                    for j in range(nb):
                        nc.tensor.matmul(
                            out=ps[:], lhsT=ident_b[:], rhs=gb[:, j, :],
                            start=(j == 0), stop=(j == nb - 1),
                        )
                    nc.vector.tensor_add(o1_all[:, w, :], ps[:], o1A[:, w, :])
                    nc.vector.tensor_scalar_mul(
                        o1_all[:, w, :], o1_all[:, w, :], dismy_s[:, w: w + 1]
                    )
                    sq = sqpool.tile([P, D1], bf16, tag="sq")
                    nc.vector.tensor_mul(sq[:], o1_all[:, w, :], o1_all[:, w, :])
                    mcol = mask_b[:, 1:2] if w == WPC - 1 else mask_b[:, 0:1]
                    nc.tensor.matmul(
                        out=st_s[:], lhsT=mcol, rhs=o1_all[:, w, :],
                        start=(w == 0), stop=(w == WPC - 1), skip_group_check=True,
                    )
                    nc.tensor.matmul(
                        out=st_q[:], lhsT=mcol, rhs=sq[:],
                        start=(w == 0), stop=(w == WPC - 1), skip_group_check=True,
                    )
                stats1 = o1_pool.tile([1, 2 * D1], f32)
                nc.vector.tensor_copy(out=stats1[:, 0:D1], in_=st_s[:])
                nc.vector.tensor_copy(out=stats1[:, D1:], in_=st_q[:])
            if phases < 3:
                outst = cpool.tile([P, WPC, D2], f32)
                nc.vector.tensor_copy(out=outst[:], in_=o1_all[:, :, 0:D2])
                nc.gpsimd.dma_start(
                    out=out_d[0:SLOTS, :].rearrange("(t p) d -> p t d", p=P),
                    in_=outst[:],
                )
                mid_es.close()
                return nc
            nc.gpsimd.dma_start(out=ar1_in[:, :], in_=stats1[:])
            nc.gpsimd.collective_compute(
                "AllReduce", ALU.add,
                ins=[ar1_in[:, :]], outs=[ar1_out[:, :]], replica_groups=RG,
            )

            # ---------------- BN1 factors + h2 shard ----------------
            bnp = mid_es.enter_context(tc.tile_pool(name="bn1", bufs=1))
            # o1 transposes run during the stats AllReduce (depend only on o1_all)
            o1T = bnp.tile([P, WPC, KC2, P], bf16)
            with tc.tile_pool(name="trh", bufs=4, space="PSUM") as trh:
                for w in range(WPC):
                    for c in range(KC2):
                        tpp = trh.tile([P, P], bf16, tag="tr0")
                        nc.tensor.transpose(
                            out=tpp[:], in_=o1_all[:, w, c * P: (c + 1) * P],
                            identity=ident_b[:],
                        )
                        nc.vector.tensor_copy(out=o1T[:, w, c, :], in_=tpp[:])
            sg = bnp.tile([1, 2 * D1], f32)
            nc.sync.dma_start(out=sg[:], in_=ar1_out[:, :])
            mean = bnp.tile([1, D1], f32)
            nc.vector.tensor_scalar_mul(mean[:], sg[:, 0:D1], 1.0 / N)
            ex2 = bnp.tile([1, D1], f32)
            nc.vector.tensor_scalar_mul(ex2[:], sg[:, D1:], 1.0 / N)
            var = bnp.tile([1, D1], f32)
            nc.vector.tensor_mul(var[:], mean[:], mean[:])
            nc.vector.tensor_sub(var[:], ex2[:], var[:])
            epst = bnp.tile([1, 1], f32)
            nc.vector.memset(epst[:], EPS)
            sd = bnp.tile([1, D1], f32)
            nc.scalar.activation(sd[:], var[:], AF.Sqrt, bias=epst[:])
            rstd = bnp.tile([1, D1], f32)
            nc.vector.reciprocal(rstd[:], sd[:])
            a1 = bnp.tile([1, D1], f32)
            nc.vector.tensor_mul(a1[:], rstd[:], gb_s[:, 0:D1])
            c1 = bnp.tile([1, D1], f32)
            nc.vector.tensor_mul(c1[:], mean[:], a1[:])
            nc.vector.tensor_sub(c1[:], gb_s[:, D1: 2 * D1], c1[:])
            # transpose (a1, c1) -> per-partition chunks [128, 2] per KC2 chunk
            acT = bnp.tile([P, KC2, 2], f32)
            with tc.tile_pool(name="trp", bufs=4, space="PSUM") as trpool:
                for c in range(KC2):
                    tpa = trpool.tile([P, 1], f32, tag="tra")
                    nc.tensor.transpose(
                        out=tpa[:], in_=a1[:, c * P: (c + 1) * P],
                        identity=ident_f[0:1, 0:1],
                    )
                    nc.vector.tensor_copy(out=acT[:, c, 0:1], in_=tpa[:])
                    tpc = trpool.tile([P, 1], f32, tag="trc")
                    nc.tensor.transpose(
                        out=tpc[:], in_=c1[:, c * P: (c + 1) * P],
                        identity=ident_f[0:1, 0:1],
                    )
                    nc.vector.tensor_copy(out=acT[:, c, 1:2], in_=tpc[:])

            # per window: transpose o1 chunk, BN+ReLU (split ACT/DVE), W2 matmul
            h2stage = bnp.tile([P, WPC, D2], bf16)
            with (
                tc.tile_pool(name="bnr", bufs=4) as bpool,
                tc.tile_pool(name="h2p", bufs=2, space="PSUM") as h2pool,
                tc.tile_pool(name="trq", bufs=4, space="PSUM") as trq,
            ):
                for w in range(WPC):
                    h2ps = h2pool.tile([P, D2], f32, tag="h2ps")
                    for c in range(KC2):
                        bnr = bpool.tile([P, P], bf16, tag="bnr")
                        if w % 2 == 0:
                            nc.scalar.activation(
                                bnr[:], o1T[:, w, c, :], AF.Relu,
                                bias=acT[:, c, 1:2], scale=acT[:, c, 0:1],
                            )
                        else:
                            nc.vector.tensor_scalar(
                                out=bnr[:], in0=o1T[:, w, c, :],
                                scalar1=acT[:, c, 0:1], scalar2=acT[:, c, 1:2],
                                op0=ALU.mult, op1=ALU.add,
                            )
                            nc.vector.tensor_scalar_max(bnr[:], bnr[:], 0.0)
                        nc.tensor.matmul(
                            out=h2ps[:], lhsT=bnr[:], rhs=w2_s[:, c, :],
                            start=(c == 0), stop=(c == KC2 - 1),
                        )
                    nc.vector.tensor_scalar_mul(
                        h2stage[:, w, :], h2ps[:], dismy_s[:, w: w + 1]
                    )
            zrow2 = bnp.tile([1, D2], bf16)
            nc.vector.memset(zrow2[:], 0)
            nc.gpsimd.dma_start(
                out=h2shard[0:SLOTS, :].rearrange("(t p) d -> p t d", p=P),
                in_=h2stage[:],
            )
            nc.gpsimd.dma_start(out=h2shard[SLOTS:BLK, :], in_=zrow2[:])
            nc.gpsimd.collective_compute(
                "AllGather", ALU.bypass,
                ins=[h2shard[:, :]], outs=[h2tab[:, :]], replica_groups=RG,
            )
            if phases < 4:
                outst = cpool.tile([P, WPC, D2], f32)
                nc.vector.tensor_copy(out=outst[:], in_=h2stage[:])
                nc.gpsimd.dma_start(
                    out=out_d[0:SLOTS, :].rearrange("(t p) d -> p t d", p=P),
                    in_=outst[:],
                )
                mid_es.close()
                return nc
            mid_es.close()

            # ---------------- conv2 (A+B into one PSUM per window) -----------
            o2_pool = es.enter_context(tc.tile_pool(name="o2", bufs=1))
            o2_all = o2_pool.tile([P, WPC, D2], f32)
            viewA2 = h2tab[ds(parity * HB, HB), :]
            viewB2 = h2tab[ds(HB - parity * HB, HB), :]
            with (
                tc.tile_pool(name="g2", bufs=3) as gpool2,
                tc.tile_pool(name="sq2", bufs=2) as sqpool2,
                tc.tile_pool(name="c2p", bufs=3, space="PSUM") as wpool2,
                tc.tile_pool(name="st2p", bufs=1, space="PSUM") as stpool2,
            ):
                st2_s = stpool2.tile([1, D2], f32, tag="st2_s")
                st2_q = stpool2.tile([1, D2], f32, tag="st2_q")
                for w in range(WPC):
                    na, nb = int(NA[w]), int(NB[w])
                    nt = na + nb
                    gb = gpool2.tile([P, NAmax + NBmax, D2], bf16, tag="g2")
                    nc.gpsimd.dma_gather(
                        gb[:, 0:na, :], viewA2,
                        idx_s[:, offA[w] // 16: offA[w] // 16 + na * 8],
                        na * P, na * P, D2, elem_step=D2, single_packet=False,
                    )
                    nc.gpsimd.dma_gather(
                        gb[:, na:nt, :], viewB2,
                        idx_s[:, offB[w] // 16: offB[w] // 16 + nb * 8],
                        nb * P, nb * P, D2, elem_step=D2, single_packet=False,
                    )
                    ps = wpool2.tile([P, D2], f32, tag="win2")
                    for j in range(nt):
                        nc.tensor.matmul(
                            out=ps[:], lhsT=ident_b[:], rhs=gb[:, j, :],
                            start=(j == 0), stop=(j == nt - 1),
                        )
                    nc.vector.tensor_scalar_mul(
                        o2_all[:, w, :], ps[:], dismy_s[:, w: w + 1]
                    )
                    o2b = sqpool2.tile([P, D2], bf16, tag="o2b")
                    nc.vector.tensor_copy(out=o2b[:], in_=o2_all[:, w, :])
                    sq = sqpool2.tile([P, D2], bf16, tag="sq2")
                    nc.vector.tensor_mul(sq[:], o2_all[:, w, :], o2_all[:, w, :])
                    mcol = mask_b[:, 1:2] if w == WPC - 1 else mask_b[:, 0:1]
                    nc.tensor.matmul(
                        out=st2_s[:], lhsT=mcol, rhs=o2b[:],
                        start=(w == 0), stop=(w == WPC - 1), skip_group_check=True,
                    )
                    nc.tensor.matmul(
                        out=st2_q[:], lhsT=mcol, rhs=sq[:],
                        start=(w == 0), stop=(w == WPC - 1), skip_group_check=True,
                    )
                stats2 = o2_pool.tile([1, 2 * D2], f32)
                nc.vector.tensor_copy(out=stats2[:, 0:D2], in_=st2_s[:])
                nc.vector.tensor_copy(out=stats2[:, D2:], in_=st2_q[:])
            if phases < 5:
                outst = cpool.tile([P, WPC, D2], f32)
                nc.vector.tensor_copy(out=outst[:], in_=o2_all[:, :, :])
                nc.gpsimd.dma_start(
                    out=out_d[0:SLOTS, :].rearrange("(t p) d -> p t d", p=P),
                    in_=outst[:],
                )
                return nc
            nc.gpsimd.dma_start(out=ar2_in[:, :], in_=stats2[:])
            nc.gpsimd.collective_compute(
                "AllReduce", ALU.add,
                ins=[ar2_in[:, :]], outs=[ar2_out[:, :]], replica_groups=RG,
            )

            # ---------------- BN2 + output ----------------
            sg2 = o2_pool.tile([1, 2 * D2], f32)
            nc.sync.dma_start(out=sg2[:], in_=ar2_out[:, :])
            mean2 = o2_pool.tile([1, D2], f32)
            nc.vector.tensor_scalar_mul(mean2[:], sg2[:, 0:D2], 1.0 / N)
            ex22 = o2_pool.tile([1, D2], f32)
            nc.vector.tensor_scalar_mul(ex22[:], sg2[:, D2:], 1.0 / N)
            var2 = o2_pool.tile([1, D2], f32)
            nc.vector.tensor_mul(var2[:], mean2[:], mean2[:])
            nc.vector.tensor_sub(var2[:], ex22[:], var2[:])
            epst2 = o2_pool.tile([1, 1], f32)
            nc.vector.memset(epst2[:], EPS)
            sd2 = o2_pool.tile([1, D2], f32)
            nc.scalar.activation(sd2[:], var2[:], AF.Sqrt, bias=epst2[:])
            rstd2 = o2_pool.tile([1, D2], f32)
            nc.vector.reciprocal(rstd2[:], sd2[:])
            a2 = o2_pool.tile([1, D2], f32)
            nc.vector.tensor_mul(a2[:], rstd2[:], gb_s[:, 2 * D1: 2 * D1 + D2])
            c2 = o2_pool.tile([1, D2], f32)
            nc.vector.tensor_mul(c2[:], mean2[:], a2[:])
            nc.vector.tensor_sub(c2[:], gb_s[:, 2 * D1 + D2:], c2[:])

            # broadcast a2/c2 across partitions via ones-column matmul
            onesrow = o2_pool.tile([1, P], f32)
            nc.vector.memset(onesrow[:], 1.0)
            a2b = o2_pool.tile([P, D2], f32)
            c2b = o2_pool.tile([P, D2], f32)
            with tc.tile_pool(name="bn2p", bufs=2, space="PSUM") as bn2p:
                bps = bn2p.tile([P, D2], f32, tag="b2a")
                nc.tensor.matmul(out=bps[:], lhsT=onesrow[:], rhs=a2[:], start=True, stop=True)
                nc.vector.tensor_copy(out=a2b[:], in_=bps[:])
                cps = bn2p.tile([P, D2], f32, tag="b2c")
                nc.tensor.matmul(out=cps[:], lhsT=onesrow[:], rhs=c2[:], start=True, stop=True)
                nc.vector.tensor_copy(out=c2b[:], in_=cps[:])

            outst = o2_pool.tile([P, WPC, D2], f32)
            for w in range(WPC):
                nc.vector.tensor_mul(outst[:, w, :], o2_all[:, w, :], a2b[:])
                nc.vector.tensor_add(outst[:, w, :], outst[:, w, :], c2b[:])
            nc.gpsimd.dma_start(
                out=out_d[0:SLOTS, :].rearrange("(t p) d -> p t d", p=P),
                in_=outst[:],
            )

    return nc


# ---------------------------------------------------------------- entry point

def _run(x, edge_index, W1, gamma1, beta1, W2, gamma2, beta2, cfg, trace=False):
    from concourse.bass_utils import run_bass_kernel_spmd

    N = cfg["N"]
    pp = _preprocess(edge_index, N)
    xb, w1b, w2b = _pack_inputs(np.asarray(x, np.float32), np.asarray(W1, np.float32),
                                np.asarray(W2, np.float32), pp, cfg)
    nc = _build_kernel(cfg, pp, phases=int(__import__("os").environ.get("K_PHASES", "5")))
    nc.compile()

    NT2 = pp["NTILES"] // 2
    shared = {
        "w1b": np.ascontiguousarray(w1b),
        "w2b": np.ascontiguousarray(w2b),
        "statmask": np.ascontiguousarray(pp["statmask"]),
        "gamma1": np.asarray(gamma1, np.float32).reshape(1, -1),
        "beta1": np.asarray(beta1, np.float32).reshape(1, -1),
        "gamma2": np.asarray(gamma2, np.float32).reshape(1, -1),
        "beta2": np.asarray(beta2, np.float32).reshape(1, -1),
    }
    xb_lo = np.ascontiguousarray(xb[:NT2])
    xb_hi = np.ascontiguousarray(xb[NT2:])
    in_maps = []
    for c in range(N_CORES):
        m = dict(shared)
        m["xb"] = xb_lo if c % 2 == 0 else xb_hi
        m["idx"] = np.ascontiguousarray(pp["idx_wrapped"][c])
        m["dismy"] = np.ascontiguousarray(pp["dismy"][c])
        in_maps.append(m)

    res = run_bass_kernel_spmd(nc, in_maps, core_ids=list(range(N_CORES)), trace=trace)
    _run.last_nc = nc

    D2 = cfg["D2"]
    out = np.empty((N, D2), np.float32)
    pos, core_of = pp["pos"], pp["core_of"]
    for c in range(N_CORES):
        nodes = np.flatnonzero(core_of == c)
        out[nodes] = res.results[c]["out"][pos[nodes]]
    _run.last_result = res
    return out


def kernel(x, edge_index, W1, b1, gamma1, beta1, W2, b2, gamma2, beta2):
    # b1/b2 cancel exactly through BatchNorm's mean subtraction; unused.
    return _run(x, edge_index, W1, gamma1, beta1, W2, gamma2, beta2, _FULL_CFG)


# revision 11
# speedup vs baseline: 1.0583x; 1.0318x over previous
"""GCN encoder (2x GCNConv + BatchNorm + ReLU) on 8 Trainium2 NeuronCores.

Strategy (graph/data parallel, per sharding hint):
- Nodes are sharded across the 8 cores; each core owns 49 "windows" of 128
  destination nodes.  Source nodes are split into a "lo" half (owned by cores
  0-3) and "hi" half (cores 4-7); the half assignment is optimized on the host
  (discrepancy balancing) so each destination's in-edges split ~evenly, which
  makes the per-window gather-chunk maxima tight (low padding waste).
- norm factorizes: norm(s,d) = dis[s]*dis[d].  Source scaling dis[s] is folded
  into the feature tables (h~ = dis * h); destination scaling dis[d] is applied
  on eviction.  Messages aggregate with a constant identity matmul into PSUM.
- Gathers use the int16 dma_gather embedding path; each window has an "A"
  segment (sources in the half this core's HBM-pair parity built) and a "B"
  segment (other half); pad slots point at an all-zero row.
- Phase 1 (h1 = dis*(x @ W1)) is split across HBM-pair cores: the pair shares
  one h1 table (addr_space="Shared"); the even core computes/writes the lo
  half, the odd core the hi half (rank-dependent write offsets via
  partition_id + DynSlice).  A 2-core AllReduce barrier syncs the pair; it is
  hidden behind conv1's "A" pass, which only reads the self-built half.
- h2 = relu(bn1(conv1)) @ W2 is computed per-shard; an AllGather replicates
  the h2 table for conv2.  BatchNorm stats use E[x^2]-mean^2 via ones-vector
  matmuls accumulated in PSUM, then AllReduce.  b1/b2 cancel in BN.
"""

import sys

sys.path.insert(0, "/opt/trn_rl_repo")

import numpy as np

N_CORES = 8
P = 128
EPS = 1e-5

_FULL_CFG = dict(N=50000, IN=512, D1=256, D2=128)


# ---------------------------------------------------------------- host preprocessing

def _balance_halves(S, D, deg, N, seed=0):
    """Assign each node to the lo (+1) or hi (-1) half so that every dst's
    in-edges split ~evenly between halves.  Greedy discrepancy minimization."""
    order_s = np.argsort(S, kind="stable")
    Ds = D[order_s]
    starts = np.searchsorted(S[order_s], np.arange(N + 1))
    outdeg = np.diff(starts)
    rng = np.random.default_rng(seed)
    h = np.where(rng.random(N) < 0.5, 1, -1)

    for r in range(120):
        cur = np.bincount(D, weights=h[S].astype(np.float64), minlength=N)
        s_cursum = np.add.reduceat(cur[Ds], starts[:-1]) if len(Ds) else np.zeros(N)
        s_cursum[outdeg == 0] = 0
        gain = h * s_cursum - outdeg
        batch = 3000 if r < 10 else (800 if r < 40 else 250)
        lo_c = np.flatnonzero((gain > 0) & (h == 1))
        hi_c = np.flatnonzero((gain > 0) & (h == -1))
        nlo = int((h == 1).sum())
        k_lo = min(len(lo_c), batch + max(0, nlo - N // 2))
        k_hi = min(len(hi_c), batch + max(0, N // 2 - nlo))
        if k_lo + k_hi == 0:
            break
        h[lo_c[np.argsort(-gain[lo_c])[:k_lo]]] = -1
        h[hi_c[np.argsort(-gain[hi_c])[:k_hi]]] = 1

    cur = np.bincount(D, weights=h[S].astype(np.float64), minlength=N).astype(np.int64)
    in_order = np.argsort(D, kind="stable")
    Sin = S[in_order]
    in_starts = np.searchsorted(D[in_order], np.arange(N + 1))
    for sweep in range(6):
        bad = np.flatnonzero(np.abs(cur) >= 3)
        bad = bad[np.argsort(-np.abs(cur[bad]))]
        if len(bad) == 0:
            break
        for d in bad:
            cd = cur[d]
            if abs(cd) < 3:
                continue
            sign = 1 if cd > 0 else -1
            nbrs = Sin[in_starts[d]:in_starts[d + 1]]
            cands = nbrs[h[nbrs] == sign]
            if len(cands) == 0:
                continue
            best, bestg = -1, -(10 ** 9)
            for s in cands[:12]:
                od = Ds[starts[s]:starts[s + 1]]
                g = h[s] * cur[od].sum() - len(od)
                if g > bestg:
                    best, bestg = s, g
            od = Ds[starts[best]:starts[best + 1]]
            cur[od] -= 2 * h[best]
            h[best] = -h[best]
    # exact 50/50 split
    nlo = int((h == 1).sum())
    if nlo != N // 2:
        d = 1 if nlo > N // 2 else -1
        side = np.flatnonzero(h == d)
        s_cursum = np.add.reduceat(cur[Ds], starts[:-1]) if len(Ds) else np.zeros(N)
        s_cursum[outdeg == 0] = 0
        gain = h * s_cursum - outdeg
        take = side[np.argsort(-gain[side])[: abs(nlo - N // 2)]]
        for s in take:
            od = Ds[starts[s]:starts[s + 1]]
            cur[od] -= 2 * h[s]
            h[s] = -h[s]
    return h


def _preprocess(edge_index, N):
    """Graph preprocessing: half balancing, node permutation, A/B segment
    chunk assignment, gather indices.  Pure integer work on the host."""
    src = np.asarray(edge_index[0], dtype=np.int64)
    dst = np.asarray(edge_index[1], dtype=np.int64)
    loop = np.arange(N, dtype=np.int64)
    S = np.concatenate([src, loop])
    D = np.concatenate([dst, loop])

    deg = np.bincount(D, minlength=N)  # >= 1 (self loop)
    dis = (1.0 / np.sqrt(deg.astype(np.float64))).astype(np.float32)

    real_pc = N // N_CORES
    WPC = (real_pc + P - 1) // P          # windows per core
    SLOTS = WPC * P                        # slot positions per core
    BLK = SLOTS + 1                        # +1 trailing zero row per core block

    # side assignment (side p = nodes owned by parity-p cores), then deal
    # each side by degree to its 4 cores
    hsplit = _balance_halves(S, D, deg, N)
    core_of = np.empty(N, dtype=np.int64)
    for par, hv in ((0, 1), (1, -1)):
        nodes = np.flatnonzero(hsplit == hv)
        o = nodes[np.argsort(-deg[nodes], kind="stable")]
        core_of[o] = (np.arange(len(o)) % 4) * 2 + par

    srcpar = core_of[S] % 2
    deg_lo = np.bincount(D[srcpar == 0], minlength=N)  # side-0 in-degree
    deg_hi = deg - deg_lo                               # side-1 in-degree

    # position within core: sort by max(lo,hi) desc (then total) — with the
    # balanced halves lo≈hi, this keeps BOTH per-window maxima tight
    pos = np.empty(N, dtype=np.int64)
    node_by_cp = np.full((N_CORES, SLOTS), -1, dtype=np.int64)
    for c in range(N_CORES):
        nodes_c = np.flatnonzero(core_of == c)
        key = np.maximum(deg_lo[nodes_c], deg_hi[nodes_c])
        o = np.lexsort((-(deg_lo[nodes_c] + deg_hi[nodes_c]), -key))
        snodes = nodes_c[o]
        pos[snodes] = np.arange(len(snodes))
        node_by_cp[c, : len(snodes)] = snodes

    # per-core per-window maxima of lo/hi degree
    dlo_cp = np.zeros((N_CORES, SLOTS), dtype=np.int64)
    dhi_cp = np.zeros((N_CORES, SLOTS), dtype=np.int64)
    m = node_by_cp >= 0
    dlo_cp[m] = deg_lo[node_by_cp[m]]
    dhi_cp[m] = deg_hi[node_by_cp[m]]
    WL = dlo_cp.reshape(N_CORES, WPC, P).max(axis=2)  # [core, w]
    WH = dhi_cp.reshape(N_CORES, WPC, P).max(axis=2)
    ev = np.arange(N_CORES) % 2 == 0
    # segment A = this core's own side (== the half its pair-parity built)
    NA = np.maximum(WL[ev].max(axis=0), WH[~ev].max(axis=0))
    NB = np.maximum(WH[ev].max(axis=0), WL[~ev].max(axis=0))

    # idx segment offsets: per window [A seg][B seg], chunk-major inside
    seg = (NA + NB) * P
    base = np.concatenate([[0], np.cumsum(seg)])
    offA = base[:-1]
    offB = base[:-1] + NA * P
    TOT = int(base[-1])

    # edge -> segment: A iff src side parity == dst-core parity
    cD = core_of[D]
    inA = (core_of[S] % 2) == (cD % 2)
    key = D * 2 + (~inA).astype(np.int64)
    ksort = np.argsort(key, kind="stable")
    skey = key[ksort]
    starts = np.concatenate([[0], np.flatnonzero(np.diff(skey)) + 1])
    group_len = np.diff(np.concatenate([starts, [len(skey)]]))
    chunk_sorted = np.arange(len(skey)) - np.repeat(starts, group_len)
    chunk = np.empty(len(S), dtype=np.int64)
    chunk[ksort] = chunk_sorted

    wD = pos[D] // P
    slotD = pos[D] % P
    rel = (core_of[S] // 2) * BLK + pos[S]  # within-side row
    assert rel.max() < 32768
    epos = np.where(inA, offA[wD], offB[wD]) + chunk * P + slotD

    PADIDX = SLOTS  # block 0's trailing zero row (within-half view)
    flat = np.full(N_CORES * TOT, PADIDX, dtype=np.int16)
    flat[cD * TOT + epos] = rel.astype(np.int16)
    flat = flat.reshape(N_CORES, TOT)
    # wrap: idx i -> [i%16, i//16], replicated across the 8 groups of 16 rows
    wrapped16 = flat.reshape(N_CORES, TOT // 16, 16).transpose(0, 2, 1)
    idx_wrapped = np.tile(wrapped16, (1, P // 16, 1))  # [cores, 128, TOT/16]

    # per-core dis (by slot), 1.0 for dummies
    dis_cp = np.ones((N_CORES, SLOTS), dtype=np.float32)
    dis_cp[m] = dis[node_by_cp[m]]
    dismy = dis_cp.reshape(N_CORES, WPC, P).transpose(0, 2, 1)  # [c, 128, WPC]

    NTILES = N_CORES * WPC

    # stats mask: last window has (SLOTS - real_pc) dummy rows at the end
    n_dummy = SLOTS - real_pc
    statmask = np.ones((P, 2), dtype=np.float32)
    if n_dummy:
        statmask[P - n_dummy:, 1] = 0.0

    waste = float(seg.sum()) / max(1, len(S) / N_CORES) - 1.0
    return dict(
        WPC=WPC, SLOTS=SLOTS, BLK=BLK, NTILES=NTILES,
        NA=NA.astype(int), NB=NB.astype(int), TOT=TOT,
        offA=offA, offB=offB,
        idx_wrapped=idx_wrapped, dismy=dismy,
        statmask=statmask, node_by_cp=node_by_cp, pos=pos, core_of=core_of,
        dis=dis, real_pc=real_pc, waste=waste,
    )


def _pack_inputs(x, W1, W2, pp, cfg):
    """Build the device input arrays.  xb is the full permuted/prescaled x in
    table order; each core receives only the half it builds in phase 1."""
    import ml_dtypes

    bf16 = ml_dtypes.bfloat16
    N, IN, D1, D2 = cfg["N"], cfg["IN"], cfg["D1"], cfg["D2"]
    SLOTS, NTILES = pp["SLOTS"], pp["NTILES"]
    KC = IN // P

    corder = [0, 2, 4, 6, 1, 3, 5, 7]  # table block b -> owning core
    nbc = pp["node_by_cp"][corder]      # [block, SLOTS] in table order
    xperm = np.zeros((N_CORES * SLOTS, IN), dtype=np.float32)
    m = nbc >= 0
    xperm[np.flatnonzero(m.reshape(-1))] = (
        x[nbc[m]] * pp["dis"][nbc[m]][:, None]
    )
    xb = (
        xperm.reshape(NTILES, P, KC, P)   # [b, j, kc, p]
        .transpose(0, 3, 2, 1)            # [b, p, kc, j]
        .astype(bf16)
    )
    w1b = W1.reshape(KC, P, D1).transpose(1, 0, 2).astype(bf16)   # [p, kc, D1]
    w2b = W2.reshape(D1 // P, P, D2).transpose(1, 0, 2).astype(bf16)  # [p, kc, D2]
    return xb, w1b, w2b


# ---------------------------------------------------------------- device kernel

def _build_kernel(cfg, pp, phases=5):
    import concourse.bacc as bacc
    import concourse.mybir as mybir
    import concourse.tile as tile
    from concourse.masks import make_identity
    from concourse.bass import ds
    from contextlib import ExitStack

    N, IN, D1, D2 = cfg["N"], cfg["IN"], cfg["D1"], cfg["D2"]
    WPC, SLOTS, BLK, NTILES = pp["WPC"], pp["SLOTS"], pp["BLK"], pp["NTILES"]
    NA, NB, TOT = pp["NA"], pp["NB"], pp["TOT"]
    offA, offB = pp["offA"], pp["offB"]
    KC = IN // P
    KC2 = D1 // P
    HB = (N_CORES // 2) * BLK        # rows per half
    NROWS = N_CORES * BLK            # table rows
    NBLK = N_CORES // 2              # blocks built per core (its parity half)
    NTILES2 = NBLK * WPC             # xb tiles per core
    NAmax = int(NA.max())
    NBmax = int(NB.max())
    RG = [list(range(N_CORES))]
    RG4 = [list(range(0, N_CORES, 2)), list(range(1, N_CORES, 2))]
    PAIR_RG = [[2 * k, 2 * k + 1] for k in range(N_CORES // 2)]
    f32, bf16, i16 = mybir.dt.float32, mybir.dt.bfloat16, mybir.dt.int16
    AF = mybir.ActivationFunctionType
    ALU = mybir.AluOpType

    nc = bacc.Bacc(num_devices=N_CORES)

    # ---- I/O
    xb_d = nc.dram_tensor("xb", [NTILES2, P, KC, P], bf16, kind="ExternalInput")
    w1_d = nc.dram_tensor("w1b", [P, KC, D1], bf16, kind="ExternalInput")
    w2_d = nc.dram_tensor("w2b", [P, KC2, D2], bf16, kind="ExternalInput")
    idx_d = nc.dram_tensor("idx", [P, TOT // 16], i16, kind="ExternalInput")
    dismy_d = nc.dram_tensor("dismy", [P, WPC], f32, kind="ExternalInput")
    mask_d = nc.dram_tensor("statmask", [P, 2], f32, kind="ExternalInput")
    g1_d = nc.dram_tensor("gamma1", [1, D1], f32, kind="ExternalInput")
    b1_d = nc.dram_tensor("beta1", [1, D1], f32, kind="ExternalInput")
    g2_d = nc.dram_tensor("gamma2", [1, D2], f32, kind="ExternalInput")
    b2_d = nc.dram_tensor("beta2", [1, D2], f32, kind="ExternalInput")
    out_d = nc.dram_tensor("out", [SLOTS, D2], f32, kind="ExternalOutput")

    # ---- internal DRAM
    # h1tab is shared within an HBM core pair: the even core writes the lo
    # half, the odd core the hi half.
    h1tab = nc.dram_tensor("h1tab", [NROWS, D1], bf16, kind="Internal",
                           addr_space="Shared")
    h2shard = nc.dram_tensor("h2shard", [BLK, D2], bf16, kind="Internal")
    h2mine = nc.dram_tensor("h2mine", [HB, D2], bf16, kind="Internal")
    h2pair = nc.dram_tensor("h2pair", [NROWS, D2], bf16, kind="Internal", addr_space="Shared")
    bar2_in = nc.dram_tensor("bar2_in", [1, 4], bf16, kind="Internal")
    bar2_out = nc.dram_tensor("bar2_out", [1, 4], bf16, kind="Internal")
    bar_in = nc.dram_tensor("bar_in", [1, 4], bf16, kind="Internal")
    bar_out = nc.dram_tensor("bar_out", [1, 4], bf16, kind="Internal")
    ar1_in = nc.dram_tensor("ar1_in", [1, 2 * D1], f32, kind="Internal")
    ar1_out = nc.dram_tensor("ar1_out", [1, 2 * D1], f32, kind="Internal", addr_space="Shared")
    ar2_in = nc.dram_tensor("ar2_in", [1, 2 * D2], f32, kind="Internal")
    ar2_out = nc.dram_tensor("ar2_out", [1, 2 * D2], f32, kind="Internal", addr_space="Shared")

    import concourse.bass as bass

    def pad_rows_ap(tensor, Dd):
        # rows {c*BLK + SLOTS : c in 0..7} of a [NROWS, Dd] table
        return bass.AP(tensor, SLOTS * Dd, [[BLK * Dd, N_CORES], [1, Dd]])

    with tile.TileContext(nc) as tc:
        es = ExitStack()
        with es:
            parity = nc.gpsimd.partition_id() % 2

            cpool = es.enter_context(tc.tile_pool(name="const", bufs=1))
            ident_b = cpool.tile([P, P], bf16)
            make_identity(nc, ident_b[:])
            ident_f = cpool.tile([P, P], f32)
            make_identity(nc, ident_f[:])
            w1_s = cpool.tile([P, KC, D1], bf16)
            nc.sync.dma_start(out=w1_s[:], in_=w1_d[:, :, :])
            w2_s = cpool.tile([P, KC2, D2], bf16)
            nc.sync.dma_start(out=w2_s[:], in_=w2_d[:, :, :])
            dismy_s = cpool.tile([P, WPC], f32)
            nc.sync.dma_start(out=dismy_s[:], in_=dismy_d[:, :])
            mask_s = cpool.tile([P, 2], f32)
            nc.sync.dma_start(out=mask_s[:], in_=mask_d[:, :])
            mask_b = cpool.tile([P, 2], bf16)
            nc.vector.tensor_copy(out=mask_b[:], in_=mask_s[:])
            idx_s = cpool.tile([P, TOT // 16], i16)
            nc.sync.dma_start(out=idx_s[:], in_=idx_d[:, :])
            gb_s = cpool.tile([1, 2 * D1 + 2 * D2], f32)  # gamma1|beta1|gamma2|beta2
            nc.sync.dma_start(out=gb_s[:, 0:D1], in_=g1_d[:, :])
            nc.sync.dma_start(out=gb_s[:, D1: 2 * D1], in_=b1_d[:, :])
            nc.sync.dma_start(out=gb_s[:, 2 * D1: 2 * D1 + D2], in_=g2_d[:, :])
            nc.sync.dma_start(out=gb_s[:, 2 * D1 + D2:], in_=b2_d[:, :])

            # zero ALL pad rows (both pair cores write identical zeros: benign)
            zrow = cpool.tile([N_CORES, D1], bf16)
            nc.vector.memset(zrow[:], 0)
            nc.gpsimd.dma_start(out=pad_rows_ap(h1tab, D1), in_=zrow[:])

            # ---------------- phase 1: my parity half of h1tab ----------------
            with (
                tc.tile_pool(name="p1x", bufs=6) as xpool,
                tc.tile_pool(name="p1s", bufs=2) as spool,
                tc.tile_pool(name="p1p", bufs=4, space="PSUM") as ppool1,
            ):
                XB = 7 if WPC % 7 == 0 else 1   # x tiles per DMA
                for blk in range(NBLK):
                    stage = spool.tile([P, WPC, D1], bf16, tag="stage")
                    for tb in range(WPC // XB):
                        b0 = blk * WPC + tb * XB
                        xt = xpool.tile([P, XB, KC, P], bf16, tag="xt")
                        nc.sync.dma_start(
                            out=xt[:], in_=xb_d[b0: b0 + XB].rearrange("b p k j -> p b k j")
                        )
                        for t2 in range(XB):
                            t = tb * XB + t2
                            ps = ppool1.tile([P, D1], f32, tag="ps1")
                            for kc in range(KC):
                                nc.tensor.matmul(
                                    out=ps[:], lhsT=xt[:, t2, kc, :], rhs=w1_s[:, kc, :],
                                    start=(kc == 0), stop=(kc == KC - 1),
                                )
                            nc.scalar.activation(
                                stage[:, t, :], ps[:], AF.Copy
                            )
                    nc.gpsimd.dma_start(
                        out=h1tab[ds(parity * HB + blk * BLK, SLOTS), :].rearrange(
                            "(t p) d -> p t d", p=P
                        ),
                        in_=stage[:],
                    )

            if phases < 2:
                outst = cpool.tile([P, WPC, D2], f32)
                nc.vector.memset(outst[:], 0)
                nc.gpsimd.dma_start(
                    out=out_d[0:SLOTS, :].rearrange("(t p) d -> p t d", p=P),
                    in_=outst[:],
                )
                return nc

            # ---------------- pair barrier (issued early, hidden under A) ----
            mid_es = ExitStack()
            o1_pool = mid_es.enter_context(tc.tile_pool(name="o1", bufs=1))
            barp = mid_es.enter_context(tc.tile_pool(name="bar", bufs=1))
            bar_sb = barp.tile([1, 4], bf16)
            nc.sync.dma_start(out=bar_sb[:], in_=h1tab[0:1, 0:4])
            nc.gpsimd.dma_start(out=bar_in[:, :], in_=bar_sb[:])
            nc.gpsimd.collective_compute(
                "AllReduce", ALU.add,
                ins=[bar_in[:, :]], outs=[bar_out[:, :]], replica_groups=PAIR_RG,
            )

            # ---------------- conv1 A pass: my-half gathers (no barrier) -----
            o1A = o1_pool.tile([P, WPC, D1], bf16)     # A partials
            o1_all = o1_pool.tile([P, WPC, D1], bf16)  # combined conv1 out
            viewA1 = h1tab[ds(parity * HB, HB), :]
            viewB1 = h1tab[ds(HB - parity * HB, HB), :]
            with (
                tc.tile_pool(name="gA", bufs=3) as gpoolA,
                tc.tile_pool(name="cAp", bufs=3, space="PSUM") as wpoolA,
            ):
                for w in range(WPC):
                    na = int(NA[w])
                    gb = gpoolA.tile([P, NAmax, D1], bf16, tag="gA")
                    nc.gpsimd.dma_gather(
                        gb[:, 0:na, :], viewA1,
                        idx_s[:, offA[w] // 16: offA[w] // 16 + na * 8],
                        na * P, na * P, D1, elem_step=D1, single_packet=False,
                    )
                    ps = wpoolA.tile([P, D1], f32, tag="winA")
                    for j in range(na):
                        nc.tensor.matmul(
                            out=ps[:], lhsT=ident_b[:], rhs=gb[:, j, :],
                            start=(j == 0), stop=(j == na - 1),
                        )
                    nc.vector.tensor_copy(out=o1A[:, w, :], in_=ps[:])

            # barrier completion gate: re-zero pad rows from data that depends
            # on the collective output; B-pass gathers read ranges overlapping
            # these rows -> ordered after it
            bar_sb2 = barp.tile([1, 4], bf16)
            nc.sync.dma_start(out=bar_sb2[:], in_=bar_out[:, :])
            zdep = barp.tile([N_CORES, D1], bf16)
            nc.vector.memset(zdep[:], 0)
            nc.vector.tensor_scalar_mul(zdep[0:1, 0:4], bar_sb2[:], 0.0)
            nc.gpsimd.dma_start(out=pad_rows_ap(h1tab, D1), in_=zdep[:])

            # ---------------- conv1 B pass: other half + combine + stats -----
            with (
                tc.tile_pool(name="gB", bufs=3) as gpoolB,
                tc.tile_pool(name="sq1", bufs=2) as sqpool,
                tc.tile_pool(name="cBp", bufs=3, space="PSUM") as wpoolB,
                tc.tile_pool(name="st1p", bufs=1, space="PSUM") as stpool,
            ):
                st_s = stpool.tile([1, D1], f32, tag="st_s")
                st_q = stpool.tile([1, D1], f32, tag="st_q")
                for w in range(WPC):
                    nb = int(NB[w])
                    gb = gpoolB.tile([P, NBmax, D1], bf16, tag="gB")
                    nc.gpsimd.dma_gather(
                        gb[:, 0:nb, :], viewB1,
                        idx_s[:, offB[w] // 16: offB[w] // 16 + nb * 8],
                        nb * P, nb * P, D1, elem_step=D1, single_packet=False,
                    )
                    ps = wpoolB.tile([P, D1], f32, tag="winB")
                    for j in range(nb):
                        nc.tensor.matmul(
                            out=ps[:], lhsT=ident_b[:], rhs=gb[:, j, :],
                            start=(j == 0), stop=(j == nb - 1),
                        )
                    nc.vector.tensor_add(o1_all[:, w, :], ps[:], o1A[:, w, :])
                    nc.vector.tensor_scalar_mul(
                        o1_all[:, w, :], o1_all[:, w, :], dismy_s[:, w: w + 1]
                    )
                    sq = sqpool.tile([P, D1], bf16, tag="sq")
                    nc.vector.tensor_mul(sq[:], o1_all[:, w, :], o1_all[:, w, :])
                    mcol = mask_b[:, 1:2] if w == WPC - 1 else mask_b[:, 0:1]
                    nc.tensor.matmul(
                        out=st_s[:], lhsT=mcol, rhs=o1_all[:, w, :],
                        start=(w == 0), stop=(w == WPC - 1), skip_group_check=True,
                    )
                    nc.tensor.matmul(
                        out=st_q[:], lhsT=mcol, rhs=sq[:],
                        start=(w == 0), stop=(w == WPC - 1), skip_group_check=True,
                    )
                stats1 = o1_pool.tile([1, 2 * D1], f32)
                nc.vector.tensor_copy(out=stats1[:, 0:D1], in_=st_s[:])
                nc.vector.tensor_copy(out=stats1[:, D1:], in_=st_q[:])
            if phases < 3:
                outst = cpool.tile([P, WPC, D2], f32)
                nc.vector.tensor_copy(out=outst[:], in_=o1_all[:, :, 0:D2])
                nc.gpsimd.dma_start(
                    out=out_d[0:SLOTS, :].rearrange("(t p) d -> p t d", p=P),
                    in_=outst[:],
                )
                mid_es.close()
                return nc
            nc.gpsimd.dma_start(out=ar1_in[:, :], in_=stats1[:])
            nc.gpsimd.collective_compute(
                "AllReduce", ALU.add,
                ins=[ar1_in[:, :]], outs=[ar1_out[:, :]], replica_groups=RG,
            )

            # ---------------- BN1 factors + h2 shard ----------------
            bnp = mid_es.enter_context(tc.tile_pool(name="bn1", bufs=1))
            # o1 transposes run during the stats AllReduce (depend only on o1_all)
            o1T = bnp.tile([P, WPC, KC2, P], bf16)
            with tc.tile_pool(name="trh", bufs=4, space="PSUM") as trh:
                for w in range(WPC):
                    for c in range(KC2):
                        tpp = trh.tile([P, P], bf16, tag="tr0")
                        nc.tensor.transpose(
                            out=tpp[:], in_=o1_all[:, w, c * P: (c + 1) * P],
                            identity=ident_b[:],
                        )
                        nc.vector.tensor_copy(out=o1T[:, w, c, :], in_=tpp[:])
            sg = bnp.tile([1, 2 * D1], f32)
            nc.sync.dma_start(out=sg[:], in_=ar1_out[:, :])
            mean = bnp.tile([1, D1], f32)
            nc.vector.tensor_scalar_mul(mean[:], sg[:, 0:D1], 1.0 / N)
            ex2 = bnp.tile([1, D1], f32)
            nc.vector.tensor_scalar_mul(ex2[:], sg[:, D1:], 1.0 / N)
            var = bnp.tile([1, D1], f32)
            nc.vector.tensor_mul(var[:], mean[:], mean[:])
            nc.vector.tensor_sub(var[:], ex2[:], var[:])
            epst = bnp.tile([1, 1], f32)
            nc.vector.memset(epst[:], EPS)
            sd = bnp.tile([1, D1], f32)
            nc.scalar.activation(sd[:], var[:], AF.Sqrt, bias=epst[:])
            rstd = bnp.tile([1, D1], f32)
            nc.vector.reciprocal(rstd[:], sd[:])
            a1 = bnp.tile([1, D1], f32)
            nc.vector.tensor_mul(a1[:], rstd[:], gb_s[:, 0:D1])
            c1 = bnp.tile([1, D1], f32)
            nc.vector.tensor_mul(c1[:], mean[:], a1[:])
            nc.vector.tensor_sub(c1[:], gb_s[:, D1: 2 * D1], c1[:])
            # transpose (a1, c1) -> per-partition chunks [128, 2] per KC2 chunk
            acT = bnp.tile([P, KC2, 2], f32)
            with tc.tile_pool(name="trp", bufs=4, space="PSUM") as trpool:
                for c in range(KC2):
                    tpa = trpool.tile([P, 1], f32, tag="tra")
                    nc.tensor.transpose(
                        out=tpa[:], in_=a1[:, c * P: (c + 1) * P],
                        identity=ident_f[0:1, 0:1],
                    )
                    nc.vector.tensor_copy(out=acT[:, c, 0:1], in_=tpa[:])
                    tpc = trpool.tile([P, 1], f32, tag="trc")
                    nc.tensor.transpose(
                        out=tpc[:], in_=c1[:, c * P: (c + 1) * P],
                        identity=ident_f[0:1, 0:1],
                    )
                    nc.vector.tensor_copy(out=acT[:, c, 1:2], in_=tpc[:])

            # per window: transpose o1 chunk, BN+ReLU (split ACT/DVE), W2 matmul
            h2stage = bnp.tile([P, WPC, D2], bf16)
            with (
                tc.tile_pool(name="bnr", bufs=4) as bpool,
                tc.tile_pool(name="h2p", bufs=2, space="PSUM") as h2pool,
                tc.tile_pool(name="trq", bufs=4, space="PSUM") as trq,
            ):
                for w in range(WPC):
                    h2ps = h2pool.tile([P, D2], f32, tag="h2ps")
                    for c in range(KC2):
                        bnr = bpool.tile([P, P], bf16, tag="bnr")
                        if w % 2 == 0:
                            nc.scalar.activation(
                                bnr[:], o1T[:, w, c, :], AF.Relu,
                                bias=acT[:, c, 1:2], scale=acT[:, c, 0:1],
                            )
                        else:
                            nc.vector.tensor_scalar(
                                out=bnr[:], in0=o1T[:, w, c, :],
                                scalar1=acT[:, c, 0:1], scalar2=acT[:, c, 1:2],
                                op0=ALU.mult, op1=ALU.add,
                            )
                            nc.vector.tensor_scalar_max(bnr[:], bnr[:], 0.0)
                        nc.tensor.matmul(
                            out=h2ps[:], lhsT=bnr[:], rhs=w2_s[:, c, :],
                            start=(c == 0), stop=(c == KC2 - 1),
                        )
                    nc.vector.tensor_scalar_mul(
                        h2stage[:, w, :], h2ps[:], dismy_s[:, w: w + 1]
                    )
            zrow2 = bnp.tile([1, D2], bf16)
            nc.vector.memset(zrow2[:], 0)
            nc.gpsimd.dma_start(
                out=h2shard[0:SLOTS, :].rearrange("(t p) d -> p t d", p=P),
                in_=h2stage[:],
            )
            nc.gpsimd.dma_start(out=h2shard[SLOTS:BLK, :], in_=zrow2[:])
            nc.gpsimd.collective_compute(
                "AllGather", ALU.bypass,
                ins=[h2shard[:, :]], outs=[h2mine[:, :]], replica_groups=RG4,
            )
            # ship my side to the pair-shared table for the partner's B pass
            nc.gpsimd.dma_start(
                out=h2pair[ds(parity * HB, HB), :], in_=h2mine[:, :]
            )
            bar2_sb = bnp.tile([1, 4], bf16)
            nc.sync.dma_start(out=bar2_sb[:], in_=h2pair[0:1, 0:4])
            nc.gpsimd.dma_start(out=bar2_in[:, :], in_=bar2_sb[:])
            nc.gpsimd.collective_compute(
                "AllReduce", ALU.add,
                ins=[bar2_in[:, :]], outs=[bar2_out[:, :]], replica_groups=PAIR_RG,
            )
            if phases < 4:
                outst = cpool.tile([P, WPC, D2], f32)
                nc.vector.tensor_copy(out=outst[:], in_=h2stage[:])
                nc.gpsimd.dma_start(
                    out=out_d[0:SLOTS, :].rearrange("(t p) d -> p t d", p=P),
                    in_=outst[:],
                )
                mid_es.close()
                return nc
            mid_es.close()

            # ---------------- conv2 A pass (my side, from h2mine) ------------
            o2_pool = es.enter_context(tc.tile_pool(name="o2", bufs=1))
            o2A = o2_pool.tile([P, WPC, D2], bf16)
            o2_all = o2_pool.tile([P, WPC, D2], f32)
            viewA2 = h2mine[0:HB, :]
            viewB2 = h2pair[ds(HB - parity * HB, HB), :]
            with (
                tc.tile_pool(name="g2A", bufs=3) as gpool2A,
                tc.tile_pool(name="c2Ap", bufs=3, space="PSUM") as wpool2A,
            ):
                for w in range(WPC):
                    na = int(NA[w])
                    gb = gpool2A.tile([P, NAmax, D2], bf16, tag="g2A")
                    nc.gpsimd.dma_gather(
                        gb[:, 0:na, :], viewA2,
                        idx_s[:, offA[w] // 16: offA[w] // 16 + na * 8],
                        na * P, na * P, D2, elem_step=D2, single_packet=False,
                    )
                    ps = wpool2A.tile([P, D2], f32, tag="win2A")
                    for j in range(na):
                        nc.tensor.matmul(
                            out=ps[:], lhsT=ident_b[:], rhs=gb[:, j, :],
                            start=(j == 0), stop=(j == na - 1),
                        )
                    nc.vector.tensor_copy(out=o2A[:, w, :], in_=ps[:])

            # barrier-2 gate: rewrite h2pair pad rows from bar2-dependent zeros
            bar2_sb2 = o2_pool.tile([1, 4], bf16)
            nc.sync.dma_start(out=bar2_sb2[:], in_=bar2_out[:, :])
            zdep2 = o2_pool.tile([N_CORES, D2], bf16)
            nc.vector.memset(zdep2[:], 0)
            nc.vector.tensor_scalar_mul(zdep2[0:1, 0:4], bar2_sb2[:], 0.0)
            nc.gpsimd.dma_start(out=pad_rows_ap(h2pair, D2), in_=zdep2[:])

            # ---------------- conv2 B pass (other side) + combine + stats ----
            with (
                tc.tile_pool(name="g2B", bufs=3) as gpool2B,
                tc.tile_pool(name="sq2", bufs=2) as sqpool2,
                tc.tile_pool(name="c2Bp", bufs=3, space="PSUM") as wpool2B,
                tc.tile_pool(name="st2p", bufs=1, space="PSUM") as stpool2,
            ):
                st2_s = stpool2.tile([1, D2], f32, tag="st2_s")
                st2_q = stpool2.tile([1, D2], f32, tag="st2_q")
                for w in range(WPC):
                    nb = int(NB[w])
                    gb = gpool2B.tile([P, NBmax, D2], bf16, tag="g2B")
                    nc.gpsimd.dma_gather(
                        gb[:, 0:nb, :], viewB2,
                        idx_s[:, offB[w] // 16: offB[w] // 16 + nb * 8],
                        nb * P, nb * P, D2, elem_step=D2, single_packet=False,
                    )
                    ps = wpool2B.tile([P, D2], f32, tag="win2B")
                    for j in range(nb):
                        nc.tensor.matmul(
                            out=ps[:], lhsT=ident_b[:], rhs=gb[:, j, :],
                            start=(j == 0), stop=(j == nb - 1),
                        )
                    nc.vector.tensor_add(o2_all[:, w, :], ps[:], o2A[:, w, :])
                    nc.vector.tensor_scalar_mul(
                        o2_all[:, w, :], o2_all[:, w, :], dismy_s[:, w: w + 1]
                    )
                    o2b = sqpool2.tile([P, D2], bf16, tag="o2b")
                    nc.vector.tensor_copy(out=o2b[:], in_=o2_all[:, w, :])
                    sq = sqpool2.tile([P, D2], bf16, tag="sq2")
                    nc.vector.tensor_mul(sq[:], o2_all[:, w, :], o2_all[:, w, :])
                    mcol = mask_b[:, 1:2] if w == WPC - 1 else mask_b[:, 0:1]
                    nc.tensor.matmul(
                        out=st2_s[:], lhsT=mcol, rhs=o2b[:],
                        start=(w == 0), stop=(w == WPC - 1), skip_group_check=True,
                    )
                    nc.tensor.matmul(
                        out=st2_q[:], lhsT=mcol, rhs=sq[:],
                        start=(w == 0), stop=(w == WPC - 1), skip_group_check=True,
                    )
                stats2 = o2_pool.tile([1, 2 * D2], f32)
                nc.vector.tensor_copy(out=stats2[:, 0:D2], in_=st2_s[:])
                nc.vector.tensor_copy(out=stats2[:, D2:], in_=st2_q[:])
            if phases < 5:
                outst = cpool.tile([P, WPC, D2], f32)
                nc.vector.tensor_copy(out=outst[:], in_=o2_all[:, :, :])
                nc.gpsimd.dma_start(
                    out=out_d[0:SLOTS, :].rearrange("(t p) d -> p t d", p=P),
                    in_=outst[:],
                )
                return nc
            nc.gpsimd.dma_start(out=ar2_in[:, :], in_=stats2[:])
            nc.gpsimd.collective_compute(
                "AllReduce", ALU.add,
                ins=[ar2_in[:, :]], outs=[ar2_out[:, :]], replica_groups=RG,
            )

            # ---------------- BN2 + output ----------------
            sg2 = o2_pool.tile([1, 2 * D2], f32)
            nc.sync.dma_start(out=sg2[:], in_=ar2_out[:, :])
            mean2 = o2_pool.tile([1, D2], f32)
            nc.vector.tensor_scalar_mul(mean2[:], sg2[:, 0:D2], 1.0 / N)
            ex22 = o2_pool.tile([1, D2], f32)
            nc.vector.tensor_scalar_mul(ex22[:], sg2[:, D2:], 1.0 / N)
            var2 = o2_pool.tile([1, D2], f32)
            nc.vector.tensor_mul(var2[:], mean2[:], mean2[:])
            nc.vector.tensor_sub(var2[:], ex22[:], var2[:])
            epst2 = o2_pool.tile([1, 1], f32)
            nc.vector.memset(epst2[:], EPS)
            sd2 = o2_pool.tile([1, D2], f32)
            nc.scalar.activation(sd2[:], var2[:], AF.Sqrt, bias=epst2[:])
            rstd2 = o2_pool.tile([1, D2], f32)
            nc.vector.reciprocal(rstd2[:], sd2[:])
            a2 = o2_pool.tile([1, D2], f32)
            nc.vector.tensor_mul(a2[:], rstd2[:], gb_s[:, 2 * D1: 2 * D1 + D2])
            c2 = o2_pool.tile([1, D2], f32)
            nc.vector.tensor_mul(c2[:], mean2[:], a2[:])
            nc.vector.tensor_sub(c2[:], gb_s[:, 2 * D1 + D2:], c2[:])

            # broadcast a2/c2 across partitions via ones-column matmul
            onesrow = o2_pool.tile([1, P], f32)
            nc.vector.memset(onesrow[:], 1.0)
            a2b = o2_pool.tile([P, D2], f32)
            c2b = o2_pool.tile([P, D2], f32)
            with tc.tile_pool(name="bn2p", bufs=2, space="PSUM") as bn2p:
                bps = bn2p.tile([P, D2], f32, tag="b2a")
                nc.tensor.matmul(out=bps[:], lhsT=onesrow[:], rhs=a2[:], start=True, stop=True)
                nc.vector.tensor_copy(out=a2b[:], in_=bps[:])
                cps = bn2p.tile([P, D2], f32, tag="b2c")
                nc.tensor.matmul(out=cps[:], lhsT=onesrow[:], rhs=c2[:], start=True, stop=True)
                nc.vector.tensor_copy(out=c2b[:], in_=cps[:])

            outst = o2_pool.tile([P, WPC, D2], f32)
            for w in range(WPC):
                nc.vector.tensor_mul(outst[:, w, :], o2_all[:, w, :], a2b[:])
                nc.vector.tensor_add(outst[:, w, :], outst[:, w, :], c2b[:])
            nc.gpsimd.dma_start(
                out=out_d[0:SLOTS, :].rearrange("(t p) d -> p t d", p=P),
                in_=outst[:],
            )

    return nc


# ---------------------------------------------------------------- entry point

def _run(x, edge_index, W1, gamma1, beta1, W2, gamma2, beta2, cfg, trace=False):
    from concourse.bass_utils import run_bass_kernel_spmd

    N = cfg["N"]
    pp = _preprocess(edge_index, N)
    xb, w1b, w2b = _pack_inputs(np.asarray(x, np.float32), np.asarray(W1, np.float32),
                                np.asarray(W2, np.float32), pp, cfg)
    nc = _build_kernel(cfg, pp, phases=int(__import__("os").environ.get("K_PHASES", "5")))
    nc.compile()

    NT2 = pp["NTILES"] // 2
    shared = {
        "w1b": np.ascontiguousarray(w1b),
        "w2b": np.ascontiguousarray(w2b),
        "statmask": np.ascontiguousarray(pp["statmask"]),
        "gamma1": np.asarray(gamma1, np.float32).reshape(1, -1),
        "beta1": np.asarray(beta1, np.float32).reshape(1, -1),
        "gamma2": np.asarray(gamma2, np.float32).reshape(1, -1),
        "beta2": np.asarray(beta2, np.float32).reshape(1, -1),
    }
    xb_lo = np.ascontiguousarray(xb[:NT2])
    xb_hi = np.ascontiguousarray(xb[NT2:])
    in_maps = []
    for c in range(N_CORES):
        m = dict(shared)
        m["xb"] = xb_lo if c % 2 == 0 else xb_hi
        m["idx"] = np.ascontiguousarray(pp["idx_wrapped"][c])
        m["dismy"] = np.ascontiguousarray(pp["dismy"][c])
        in_maps.append(m)

    res = run_bass_kernel_spmd(nc, in_maps, core_ids=list(range(N_CORES)), trace=trace)
    _run.last_nc = nc

    D2 = cfg["D2"]
    out = np.empty((N, D2), np.float32)
    pos, core_of = pp["pos"], pp["core_of"]
    for c in range(N_CORES):
        nodes = np.flatnonzero(core_of == c)
        out[nodes] = res.results[c]["out"][pos[nodes]]
    _run.last_result = res
    return out


def kernel(x, edge_index, W1, b1, gamma1, beta1, W2, b2, gamma2, beta2):
    # b1/b2 cancel exactly through BatchNorm's mean subtraction; unused.
    return _run(x, edge_index, W1, gamma1, beta1, W2, gamma2, beta2, _FULL_CFG)
